# revision 82
# baseline (speedup 1.0000x reference)
"""Bidirectional Mamba block (in_proj -> depthwise causal conv -> SiLU ->
forward+backward S6 selective scan -> gated combine -> out_proj) as a
Trainium2 Bass/Tile SPMD kernel over 8 NeuronCores.

Sharding: tensor-parallel over d_inner (256 channels per core). The conv and
the S6 scans are channel-independent, so they need no communication.

Collectives are pipelined per batch element:
  * AllReduce(b) of both directions' partial x-projections dbc = u @ Wx^T
    ([192, 1024] fp32) is issued as soon as batch b's conv is done; AR(b0)
    overlaps the b1 front-end and the res projection, AR(b1) overlaps the
    b0 scan phase.
  * The partial out-projection is cast to bf16 and ReduceScattered in 4
    chunks (2 per batch); the b0 chunks overlap the b1 scan phase.  (More
    or uneven chunks measured slower: each extra collective costs tens of
    microseconds of serialized launch/ring time.)

Scan layout: partitions = (8 channels x 16 states), free dim = L, one scan
instruction per (direction, batch, channel-group) on the DVE hardware scan
(fp32 internal state; the per-element feedback bubble pins it at ~2.2
cycles/element regardless of dtype).  The x transposes, in_proj and the
whole scan phase run in bf16 (rel err ~6e-3 vs the 2e-2 budget).

The entire dBu -> scan -> hC chain runs on the vector engine: dBu multiplies
the PE-produced w replica straight out of PSUM (1x rate), hC runs at the 2x
bf16 rate.  Offloading dBu/hC to GpSimd measures *slower* -- a GpSimd
tensor op co-running with a scan inflates the scan ~1.8us via SBUF port
contention, more than the offload saves.

PE array tiling: the delta/w replication matmuls have a true contraction of
8 rows, so four channel groups run concurrently in 32x128 row-tiles
(tile_position=(32i, 0), 32-row stationaries selecting 8 rows); the
per-group state reduction y = sum_n C*h has 8 output rows, so four groups
run concurrently in 128x32 column tiles (tile_position=(0, 32i)), and the
skip term u*(fD+bD) is folded into the same PSUM accumulation as a fifth,
diagonal column-tiled matmul.  Channel group g = 4i+k lives at partition
base 32i+8k, which places every concurrent quad {k, 4+k, 8+k, 12+k} in four
distinct array quadrants without any host-side channel permutation.
"""

import os
import sys

for _p in ("/opt/trn_rl_repo", "/root/.axon_site/_ro/trn_rl_repo"):
    if os.path.isdir(_p) and _p not in sys.path:
        sys.path.append(_p)

from dataclasses import dataclass

import ml_dtypes
import numpy as np

import concourse.bass as bass
import concourse.mybir as mybir
import concourse.tile as tile
from concourse import bacc

DT = mybir.dt.float32
F32R = mybir.dt.float32r
BF = mybir.dt.bfloat16
AF = mybir.ActivationFunctionType
OP = mybir.AluOpType


@dataclass(frozen=True)
class Cfg:
    n_cores: int = 8
    B: int = 2
    L: int = 1024
    M: int = 1024      # d_model
    DI: int = 2048     # d_inner
    N: int = 16        # d_state
    R: int = 64        # dt_rank
    KC: int = 4        # conv kernel

    @property
    def DC(self):  # channels per core
        return self.DI // self.n_cores

    @property
    def TOK(self):
        return self.B * self.L

    @property
    def P_CH(self):  # partitions per channel tile
        return min(128, self.DC)

    @property
    def CHT(self):  # channel tiles per core
        return self.DC // self.P_CH

    @property
    def NT(self):  # scan tiles per (dir, batch): 8 channels each
        return self.DC // 8

    @property
    def TPC(self):  # scan tiles per channel tile
        return self.P_CH // 8

    @property
    def FCH(self):  # matmul moving-dim chunk over tokens
        return min(512, self.L)

    @property
    def E(self):
        return self.R + 2 * self.N

    def check(self):
        assert self.DC % 8 == 0 and self.DC % self.P_CH == 0
        assert self.M % 128 == 0
        assert self.TOK % 128 == 0 and self.TOK % self.FCH == 0
        assert self.L % min(512, self.L) == 0
        assert self.N == 16 and self.TPC == 16


FULL = Cfg()


def build_consts(cfg: Cfg):
    """Selection matrices used as PE 'weights' (exact 0/1 values)."""
    P = 128
    ident = np.eye(P, dtype=np.float32)
    # rall32[32i+r, k, col] = 1 iff r in [8k, 8k+8) and col//16 == r-8k:
    # a 32-row stationary (32-aligned, as the PE tiling requires) that
    # replicates the 8 channel rows of group 4i+k into 128 (ch x state)
    # rows; the other 24 rows of the quadrant contribute zeros.
    rall32 = np.zeros((P, 4, P), np.float32)
    for p in range(P):
        r = p % 32
        for k in range(4):
            if 8 * k <= r < 8 * k + 8:
                rall32[p, k, (r - 8 * k) * 16:(r - 8 * k + 1) * 16] = 1.0
    # tsel[:, which, :]: out[p] = src[16*which + p%16]  (B/C replication)
    t_sel = np.zeros((2 * cfg.N, 2, P), np.float32)
    for which in range(2):
        for p in range(P):
            t_sel[cfg.N * which + p % 16, which, p] = 1.0
    # sall32[p, k, j] = 1 iff j == 8k + p//16: reduce the 16 state rows of
    # channel e into output row 8k+e of a 32-row column tile.
    sall32 = np.zeros((P, 4, 32), np.float32)
    for p in range(P):
        for k in range(4):
            sall32[p, k, 8 * k + p // 16] = 1.0
    # ddiag[p, c, i, j] = dsum[c*128+p] iff p == 32i+j: adds u*(fD+bD) into
    # the y accumulation as a 4-way column-tiled diagonal matmul.
    return ident, rall32, t_sel, sall32


def build_ddiag(cfg: Cfg, dsum_local: np.ndarray):
    P = 128
    ddiag = np.zeros((P, cfg.CHT, 4, 32), np.float32)
    for c in range(cfg.CHT):
        for p in range(P):
            i, j = p // 32, p % 32
            ddiag[p, c, i, j] = dsum_local[c * P + p]
    return ddiag.reshape(P, cfg.CHT * 4 * 32)


def build_program(cfg: Cfg) -> bass.Bass:
    cfg.check()
    P = 128
    TOK, L, M = cfg.TOK, cfg.L, cfg.M
    DC, CHT, P_CH = cfg.DC, cfg.CHT, cfg.P_CH
    FCH = cfg.FCH
    MT = M // P               # m tiles
    E, R, N = cfg.E, cfg.R, cfg.N
    LCH = L // FCH            # matmul chunks per batch
    TBB = L // P              # token blocks per batch

    nc = bacc.Bacc(
        "TRN2", target_bir_lowering=False, debug=False, num_devices=cfg.n_cores
    )

    # ---- kernel I/O ----
    x_d = nc.dram_tensor("x", [TOK, M], DT, kind="ExternalInput")
    winuT_d = nc.dram_tensor("winuT", [M, DC], BF, kind="ExternalInput")
    winrT_d = nc.dram_tensor("winrT", [M, DC], BF, kind="ExternalInput")
    wconv_d = nc.dram_tensor("wconv", [P, CHT * cfg.KC], DT, kind="ExternalInput")
    bconv_d = nc.dram_tensor("bconv", [P, CHT], DT, kind="ExternalInput")
    wxT_d = {d: nc.dram_tensor(f"wx{d}T", [DC, E], F32R, kind="ExternalInput")
             for d in "fb"}
    wdtT_d = {d: nc.dram_tensor(f"wdt{d}T", [R, DC], F32R, kind="ExternalInput")
              for d in "fb"}
    bdt_d = {d: nc.dram_tensor(f"bdt{d}", [P, CHT], DT, kind="ExternalInput")
             for d in "fb"}
    acol_d = {d: nc.dram_tensor(f"acol{d}", [P, CHT * cfg.TPC], DT,
                                kind="ExternalInput")
              for d in "fb"}
    woutT_d = nc.dram_tensor("woutT", [DC, M], BF, kind="ExternalInput")
    ident_d = nc.dram_tensor("ident", [P, P], DT, kind="ExternalInput")
    rall32_d = nc.dram_tensor("rall32", [P, 4 * P], BF, kind="ExternalInput")
    tsel_d = nc.dram_tensor("tsel", [2 * N, 2 * P], F32R, kind="ExternalInput")
    sall32_d = nc.dram_tensor("sall32", [P, 4 * 32], BF, kind="ExternalInput")
    ddiag_d = nc.dram_tensor("ddiag", [P, CHT * 4 * 32], BF,
                             kind="ExternalInput")

    out_d = nc.dram_tensor("out_rs", [TOK // cfg.n_cores, M], BF,
                           kind="ExternalOutput")

    rg = [list(range(cfg.n_cores))]
    cc_space = "Shared" if cfg.n_cores > 4 else "Local"
    # ReduceScatter chunk schedule (token offset, size): the final chunks are
    # smaller so the exposed tail after the last y is short.
    RS_CHUNKS = [(0, 512), (512, 512), (1024, 1024)]
    RS_OF_B = {0: [0, 1], 1: [2]}

    with tile.TileContext(nc) as tc:
        with tc.tile_pool(name="persist", bufs=1) as pp, \
             tc.tile_pool(name="dram", bufs=1, space="DRAM") as dp:

            # ---------- persistent SBUF (small weights) --------------------
            ident_s = pp.tile([P, P], DT)
            nc.sync.dma_start(ident_s[:], ident_d.ap())
            identb_s = pp.tile([P, P], BF)
            nc.scalar.copy(identb_s[:], ident_s[:])
            rall32_s = pp.tile([P, 4, P], BF)
            nc.sync.dma_start(rall32_s[:], rall32_d.ap().rearrange(
                "p (a b) -> p a b", a=4))
            tsel_s = pp.tile([2 * N, 2, P], F32R)
            nc.sync.dma_start(tsel_s[:], tsel_d.ap().rearrange(
                "k (a b) -> k a b", a=2))
            sall32_s = pp.tile([P, 4, 32], BF)
            nc.sync.dma_start(sall32_s[:], sall32_d.ap().rearrange(
                "p (a b) -> p a b", a=4))
            ddiag_s = pp.tile([P, CHT, 4, 32], BF)
            nc.sync.dma_start(ddiag_s[:], ddiag_d.ap().rearrange(
                "p (c a b) -> p c a b", c=CHT, a=4))
            wconv_s = pp.tile([P, CHT, cfg.KC], DT)
            nc.sync.dma_start(wconv_s[:], wconv_d.ap().rearrange(
                "p (c k) -> p c k", c=CHT))
            bconv_s = pp.tile([P, CHT], DT)
            nc.sync.dma_start(bconv_s[:], bconv_d.ap())
            wx_s, wdt_s, bdt_s, acol_s = {}, {}, {}, {}
            for d in "fb":
                wx_s[d] = pp.tile([P_CH, CHT, E], F32R, name=f"wx{d}_s")
                nc.sync.dma_start(wx_s[d][:], wxT_d[d].ap().rearrange(
                    "(c p) e -> p c e", p=P_CH))
                wdt_s[d] = pp.tile([R, DC], F32R, name=f"wdt{d}_s")
                nc.sync.dma_start(wdt_s[d][:], wdtT_d[d].ap())
                bdt_s[d] = pp.tile([P, CHT], DT, name=f"bdt{d}_s")
                nc.sync.dma_start(bdt_s[d][:], bdt_d[d].ap())
                acol_s[d] = pp.tile([P, CHT * cfg.TPC], DT, name=f"acol{d}_s")
                nc.sync.dma_start(acol_s[d][:], acol_d[d].ap())
            wout_s = pp.tile([P_CH, CHT, M], BF)
            nc.sync.dma_start(wout_s[:], woutT_d.ap().rearrange(
                "(c p) m -> p c m", p=P_CH))

            u_c = [pp.tile([P_CH, TOK], F32R, name=f"u_c{c}") for c in range(CHT)]
            u_bf = [pp.tile([P_CH, TOK], BF, name=f"u_bf{c}") for c in range(CHT)]
            sres = [pp.tile([P_CH, TOK], BF, name=f"sres{c}")
                    for c in range(CHT)]
            y_f = [pp.tile([P_CH, TOK], BF, name=f"y_f{c}") for c in range(CHT)]

            # per-batch dbc partials/reductions (both directions merged)
            dbc_part = [dp.tile([2 * E, L], DT, name=f"dbc_part{b}")
                        for b in range(cfg.B)]
            dbc_red = [dp.tile([2 * E, L], DT, addr_space=cc_space,
                               name=f"dbc_red{b}") for b in range(cfg.B)]
            out_part = [dp.tile([sz, M], BF, name=f"out_part{r}")
                        for r, (_, sz) in enumerate(RS_CHUNKS)]
            out_rs = [dp.tile([sz // cfg.n_cores, M], BF, name=f"out_rs{r}")
                      for r, (_, sz) in enumerate(RS_CHUNKS)]

            # ================= front-end (per batch) + res-proj ============
            with tc.tile_pool(name="proj", bufs=1) as jp, \
                 tc.tile_pool(name="proj_ps", bufs=1, space="PSUM") as jpp:
                xT = [jp.tile([P, TOK], BF, name=f"xT{mt}") for mt in range(MT)]
                win_s = jp.tile([P, MT, 2 * DC], BF)
                nc.sync.dma_start(win_s[:, :, :DC], winuT_d.ap().rearrange(
                    "(a p) c -> p a c", p=P))
                nc.sync.dma_start(win_s[:, :, DC:], winrT_d.ap().rearrange(
                    "(a p) c -> p a c", p=P))

                upad = [[jp.tile([P_CH, cfg.KC - 1 + L], DT,
                                 name=f"upad{c}_{b}")
                         for b in range(cfg.B)] for c in range(CHT)]

                TPG = min(4, MT)
                for b in range(cfg.B):
                    bo = b * L
                    for half in range(LCH):
                        t0 = b * TBB + half * (FCH // P)
                        for tb in range(t0, t0 + FCH // P):
                            xsb = jp.tile([P, M], DT, tag="xsb", bufs=2,
                                          name="xsb")
                            nc.sync.dma_start(
                                xsb[:], x_d.ap()[tb * P:(tb + 1) * P, :])
                            xbb = jp.tile([P, M], BF, tag="xbb", bufs=2,
                                          name="xbb")
                            nc.scalar.copy(xbb[:], xsb[:])
                            for mg in range(MT // TPG):
                                tp_ps = jpp.tile([P, TPG * P], BF, tag="tp",
                                                 bufs=4, name="tp_ps")
                                for k in range(TPG):
                                    mt = mg * TPG + k
                                    nc.tensor.transpose(
                                        tp_ps[:, k * P:(k + 1) * P],
                                        xbb[:, mt * P:(mt + 1) * P],
                                        identb_s[:])
                                for k in range(TPG):
                                    mt = mg * TPG + k
                                    nc.vector.tensor_copy(
                                        xT[mt][:, tb * P:(tb + 1) * P],
                                        tp_ps[:, k * P:(k + 1) * P])
                        f0 = half * FCH
                        for c in range(CHT):
                            ups = jpp.tile([P_CH, FCH], DT, tag="mm", bufs=4,
                                           name="ups")
                            for kt in range(MT):
                                nc.tensor.matmul(
                                    ups[:],
                                    win_s[:, kt, c * P_CH:(c + 1) * P_CH],
                                    xT[kt][:, bo + f0:bo + f0 + FCH],
                                    start=(kt == 0), stop=(kt == MT - 1))
                            nc.scalar.copy(
                                upad[c][b][:, cfg.KC - 1 + f0:
                                           cfg.KC - 1 + f0 + FCH], ups[:])

                    for c in range(CHT):
                        nc.gpsimd.memset(upad[c][b][:, :cfg.KC - 1], 0.0)
                        # depthwise causal conv + SiLU
                        acc = None
                        for k in range(cfg.KC):
                            nxt = jp.tile([P_CH, L], DT, tag="cacc",
                                          bufs=2, name="cacc")
                            tap = upad[c][b][:, k:k + L]
                            wk = wconv_s[:P_CH, c, k:k + 1]
                            if acc is None:
                                nc.vector.tensor_scalar(
                                    nxt[:], tap, wk,
                                    bconv_s[:P_CH, c:c + 1],
                                    OP.mult, OP.add)
                            else:
                                nc.vector.scalar_tensor_tensor(
                                    nxt[:], tap, wk, acc[:],
                                    OP.mult, OP.add)
                            acc = nxt
                        sg2 = jp.tile([P_CH, L], DT, tag="sg2", bufs=2,
                                      name="sg2")
                        nc.scalar.activation(sg2[:], acc[:], AF.Sigmoid)
                        nc.gpsimd.tensor_tensor(
                            u_c[c][:, bo:bo + L], acc[:], sg2[:], OP.mult)
                        nc.scalar.copy(
                            u_bf[c][:, bo:bo + L],
                            u_c[c][:, bo:bo + L].bitcast(DT))

                    # dbc partials (both dirs) for this batch -> AllReduce(b)
                    for di, d in enumerate("fb"):
                        for lh in range(LCH):
                            f0 = lh * FCH
                            bps = jpp.tile([E, FCH], DT, tag="mm", bufs=4,
                                           name="bps")
                            for c in range(CHT):
                                nc.tensor.matmul(
                                    bps[:],
                                    wx_s[d][:, c, :],
                                    u_c[c][:, bo + f0:bo + f0 + FCH],
                                    start=(c == 0), stop=(c == CHT - 1))
                            bst = jp.tile([E, FCH], DT, tag="bst", bufs=3,
                                          name="bst")
                            nc.scalar.copy(bst[:], bps[:])
                            nc.sync.dma_start(
                                dbc_part[b][di * E:(di + 1) * E,
                                            f0:f0 + FCH], bst[:])
                    nc.gpsimd.collective_compute(
                        "AllReduce", OP.add, replica_groups=rg,
                        ins=[dbc_part[b].opt()], outs=[dbc_red[b].opt()])

                # res projection (overlaps the AllReduces)
                for c in range(CHT):
                    for fc in range(TOK // FCH):
                        f0 = fc * FCH
                        rps = jpp.tile([P_CH, FCH], DT, tag="mm", bufs=4,
                                       name="rps")
                        for kt in range(MT):
                            nc.tensor.matmul(
                                rps[:],
                                win_s[:, kt, DC + c * P_CH:DC + (c + 1) * P_CH],
                                xT[kt][:, f0:f0 + FCH],
                                start=(kt == 0), stop=(kt == MT - 1))
                        sg = jp.tile([P_CH, FCH], BF, tag="sg", bufs=2,
                                     name="sg")
                        nc.scalar.activation(sg[:], rps[:], AF.Sigmoid)
                        nc.vector.tensor_tensor(sres[c][:, f0:f0 + FCH],
                                                rps[:], sg[:], OP.mult)

            # ================= scan phase + out_proj (per batch) ===========
            with tc.tile_pool(name="scan_sb", bufs=1) as sp, \
                 tc.tile_pool(name="scan_ps", bufs=1, space="PSUM") as spp, \
                 tc.tile_pool(name="comb", bufs=1) as kp:

                def rep_tile():
                    return spp.tile([P, 2 * FCH], DT, tag="rep", bufs=3,
                                    name="rep")

                for b in range(cfg.B):
                    bo = b * L
                    for di, d in enumerate("fb"):
                        off = di * E
                        dt_sb = sp.tile([R, L], F32R, tag="dt", bufs=2,
                                        name=f"dt_{d}{b}")
                        nc.sync.dma_start(
                            dt_sb[:], dbc_red[b][off:off + R, :].bitcast(F32R))
                        bc_sb = sp.tile([2 * N, L], F32R, tag="bc", bufs=2,
                                        name=f"bc_{d}{b}")
                        nc.sync.dma_start(
                            bc_sb[:],
                            dbc_red[b][off + R:off + E, :].bitcast(F32R))

                        # B/C replicated across the 8-channel groups
                        brep = sp.tile([P, L], BF, tag="brep", bufs=2,
                                       name=f"brep{d}")
                        crep = sp.tile([P, L], BF, tag="crep", bufs=2,
                                       name=f"crep{d}")
                        for which, rep in ((0, brep), (1, crep)):
                            ps = rep_tile()
                            for lh in range(LCH):
                                o = lh * FCH
                                nc.tensor.matmul(
                                    ps[:, o:o + FCH],
                                    tsel_s[:, which, :],
                                    bc_sb[:, o:o + FCH],
                                    start=True, stop=True)
                            nc.scalar.copy(rep[:], ps[:])

                        # delta = softplus(dt @ WdtT + bdt); w = delta * u
                        delta = [sp.tile([P_CH, L], BF, tag=f"delta{c}",
                                         bufs=2, name=f"delta_{d}{c}")
                                 for c in range(CHT)]
                        w_s = [sp.tile([P_CH, L], BF, tag=f"w{c}", bufs=2,
                                       name=f"w_{d}{c}") for c in range(CHT)]
                        for c in range(CHT):
                            ps = rep_tile()
                            for lh in range(LCH):
                                o = lh * FCH
                                nc.tensor.matmul(
                                    ps[:, o:o + FCH],
                                    wdt_s[d][:, c * P_CH:(c + 1) * P_CH],
                                    dt_sb[:, o:o + FCH],
                                    start=True, stop=True)
                            spt = sp.tile([P_CH, L], DT, tag="spt", bufs=2,
                                          name="spt")
                            # softplus(x + bdt) = ln(1 + exp(x + bdt))
                            nc.scalar.activation(
                                spt[:], ps[:], AF.Exp,
                                bias=bdt_s[d][:P_CH, c:c + 1])
                            nc.scalar.activation(
                                delta[c][:], spt[:], AF.Ln, bias=1.0)
                            nc.vector.tensor_tensor(
                                w_s[c][:], delta[c][:],
                                u_bf[c][:, bo:bo + L], OP.mult)

                        for c in range(CHT):
                            hC = [None] * 16
                            for k in range(4):      # concurrent quad sets
                                dA, dBu = {}, {}
                                dpq, wpq = {}, {}
                                for i in range(4):
                                    g = 4 * i + k
                                    dpq[g] = rep_tile()
                                    for lh in range(LCH):
                                        q = lh * FCH
                                        nc.tensor.matmul(
                                            dpq[g][:, q:q + FCH],
                                            rall32_s[32 * i:32 * i + 32,
                                                     k, :],
                                            delta[c][32 * i:32 * i + 32,
                                                     q:q + FCH],
                                            start=True, stop=True,
                                            tile_position=(32 * i, 0))
                                for i in range(4):
                                    g = 4 * i + k
                                    j = c * 16 + g
                                    dA[g] = sp.tile([P, L], BF, tag="dA",
                                                    bufs=6, name="dA")
                                    nc.scalar.activation(
                                        dA[g][:], dpq[g][:], AF.Exp,
                                        scale=acol_s[d][:, j:j + 1])
                                for i in range(4):
                                    g = 4 * i + k
                                    wpq[g] = rep_tile()
                                    for lh in range(LCH):
                                        q = lh * FCH
                                        nc.tensor.matmul(
                                            wpq[g][:, q:q + FCH],
                                            rall32_s[32 * i:32 * i + 32,
                                                     k, :],
                                            w_s[c][32 * i:32 * i + 32,
                                                   q:q + FCH],
                                            start=True, stop=True,
                                            tile_position=(32 * i, 0))
                                for i in range(4):
                                    g = 4 * i + k
                                    dBu[g] = sp.tile([P, L], BF, tag="dBu",
                                                     bufs=6, name="dBu")
                                    wsb = sp.tile([P, L], BF, tag="wsb",
                                                  bufs=4, name="wsb")
                                    nc.scalar.copy(wsb[:], wpq[g][:])
                                    nc.vector.tensor_tensor(
                                        dBu[g][:], wsb[:], brep[:],
                                        OP.mult)
                                for i in range(4):
                                    g = 4 * i + k
                                    h = sp.tile([P, L], BF, tag="h", bufs=10,
                                                name="h")
                                    if d == "f":
                                        nc.vector.tensor_tensor_scan(
                                            h[:], dA[g][:], dBu[g][:], 0.0,
                                            OP.mult, OP.add)
                                    else:
                                        nc.vector.tensor_tensor_scan(
                                            h[:, ::-1], dA[g][:, ::-1],
                                            dBu[g][:, ::-1],
                                            0.0, OP.mult, OP.add)
                                    hC[g] = sp.tile([P, L], BF, tag="hC",
                                                    bufs=17, name="hC")
                                    nc.vector.tensor_tensor(
                                        hC[g][:], h[:], crep[:], OP.mult)
                            # column-tiled y reduction over all 16 groups
                            y_ps = spp.tile([P_CH, L], DT, tag="y", bufs=1,
                                            name="y_ps")
                            for lh in range(LCH):
                                q = lh * FCH
                                for k in range(4):
                                    for i in range(4):
                                        g = 4 * i + k
                                        nc.tensor.matmul(
                                            y_ps[32 * i:32 * i + 32,
                                                 q:q + FCH],
                                            sall32_s[:, k, :],
                                            hC[g][:, q:q + FCH],
                                            start=(k == 0),
                                            stop=(k == 3 and d == "f"),
                                            tile_position=(0, 32 * i))
                                if d == "b":
                                    # add u*(fD+bD) into the accumulation
                                    for i in range(4):
                                        nc.tensor.matmul(
                                            y_ps[32 * i:32 * i + 32,
                                                 q:q + FCH],
                                            ddiag_s[:, c, i, :],
                                            u_bf[c][:, bo + q:bo + q + FCH],
                                            start=False, stop=True,
                                            tile_position=(0, 32 * i))
                            if d == "f":
                                nc.scalar.copy(y_f[c][:, bo:bo + L], y_ps[:])
                            else:
                                # fused combine:
                                # y = (y_f + y_b + u*(fD+bD)) * (0.5*silu(res))
                                # (the 0.5 is folded into W_out host-side)
                                ysl = y_f[c][:, bo:bo + L]
                                t1 = kp.tile([P_CH, L], BF, tag="t5", bufs=2,
                                             name="t1")
                                nc.vector.tensor_tensor(t1[:], y_ps[:],
                                                        ysl, OP.add)
                                nc.vector.tensor_tensor(
                                    ysl, t1[:], sres[c][:, bo:bo + L],
                                    OP.mult)

                    # out_proj + ReduceScatter for this batch
                    MFC = min(512, M)
                    for r in RS_OF_B[b]:
                        toff, sz = RS_CHUNKS[r]
                        for tr in range(sz // P):
                            t0 = toff + tr * P
                            ops = spp.tile([P, M], DT,
                                           tag=("rep" if b == 1 else "y"),
                                           bufs=(3 if b == 1 else 1),
                                           name="ops")
                            for mc in range(M // MFC):
                                o = mc * MFC
                                for c in range(CHT):
                                    nc.tensor.matmul(
                                        ops[:, o:o + MFC],
                                        y_f[c][:, t0:t0 + P],
                                        wout_s[:, c, o:o + MFC],
                                        start=(c == 0), stop=(c == CHT - 1))
                            ost = kp.tile([P, M], BF, tag="ost", bufs=3,
                                          name="ost")
                            nc.scalar.copy(ost[:], ops[:])
                            nc.sync.dma_start(
                                out_part[r][tr * P:(tr + 1) * P, :], ost[:])
                        nc.gpsimd.collective_compute(
                            "ReduceScatter", OP.add, replica_groups=rg,
                            ins=[out_part[r].opt()], outs=[out_rs[r].opt()])
                        nc.sync.dma_start(
                            out_d.ap()[toff // cfg.n_cores:
                                       (toff + sz) // cfg.n_cores, :],
                            out_rs[r][:])

    nc.compile()
    return nc


# --------------------------------------------------------------------------
# host side
# --------------------------------------------------------------------------

def host_prep(cfg: Cfg, inputs: dict) -> list[dict]:
    """Slice the full-model inputs into one input map per core."""
    P = 128
    f32 = np.float32

    def g(name):
        return np.asarray(inputs[name], f32)

    x = g("x").reshape(cfg.TOK, cfg.M)
    W_in = g("W_in")
    W_conv = g("W_conv").reshape(cfg.DI, cfg.KC)
    b_conv = g("b_conv")
    W_out = g("W_out")
    ident, rall32, t_sel, sall32 = build_consts(cfg)
    tsel_flat = t_sel.reshape(2 * cfg.N, 2 * P)
    sall32_flat = sall32.reshape(P, 4 * 32)
    rall32_flat = rall32.reshape(P, 4 * P)

    per = {}
    for d in "fb":
        per[d] = dict(
            A=-np.exp(g(d + "A_log")),            # (DI, N)
            D=g(d + "D"),
            Wx=g(d + "Wx"),                       # (E, DI)
            Wdt=g(d + "Wdt"),                     # (DI, R)
            bdt=g(d + "bdt"),
        )

    def col_layout(v):  # (DC,) -> (P_CH, CHT): [p, c] = v[c*P_CH + p]
        return np.ascontiguousarray(
            v.reshape(cfg.CHT, cfg.P_CH).T.astype(f32))

    def pad_p(a):  # pad partition dim up to 128
        if a.shape[0] == P:
            return np.ascontiguousarray(a.astype(f32))
        out = np.zeros((P,) + a.shape[1:], f32)
        out[:a.shape[0]] = a
        return out

    in_maps = []
    for core in range(cfg.n_cores):
        c0 = core * cfg.DC
        ch = slice(c0, c0 + cfg.DC)
        m = {
            "x": x,
            "winuT": np.ascontiguousarray(
                W_in[ch, :].T.astype(ml_dtypes.bfloat16)),
            "winrT": np.ascontiguousarray(
                W_in[cfg.DI + c0:cfg.DI + c0 + cfg.DC, :]
                .T.astype(ml_dtypes.bfloat16)),
            "wconv": pad_p(
                W_conv[ch].reshape(cfg.CHT, cfg.P_CH, cfg.KC)
                .transpose(1, 0, 2).reshape(cfg.P_CH, cfg.CHT * cfg.KC)),
            "bconv": pad_p(col_layout(b_conv[ch])),
            "woutT": np.ascontiguousarray(
                (W_out[:, ch].T * 0.5).astype(ml_dtypes.bfloat16)),
            "ident": ident,
            "ddiag": build_ddiag(
                cfg, (per["f"]["D"][ch] + per["b"]["D"][ch]).astype(f32)
            ).astype(ml_dtypes.bfloat16),
            "rall32": rall32_flat.astype(ml_dtypes.bfloat16),
            "tsel": tsel_flat,
            "sall32": sall32_flat.astype(ml_dtypes.bfloat16),
        }
        for d in "fb":
            pd = per[d]
            m[f"wx{d}T"] = np.ascontiguousarray(pd["Wx"][:, ch].T)
            m[f"wdt{d}T"] = np.ascontiguousarray(pd["Wdt"][ch, :].T)
            m[f"bdt{d}"] = pad_p(col_layout(pd["bdt"][ch]))
            # A columns: [p, j] = A[8j + p//16, p%16] (local channels)
            Ac = pd["A"][ch]                       # (DC, N)
            acol = np.empty((P, cfg.NT), f32)
            pidx = np.arange(P)
            for j in range(cfg.NT):
                acol[:, j] = Ac[8 * j + pidx // 16, pidx % 16]
            m[f"acol{d}"] = acol
        in_maps.append({k: np.ascontiguousarray(v) for k, v in m.items()})
    return in_maps


RS_CHUNKS_HOST = [(0, 512), (512, 512), (1024, 1024)]


def gather_out(cfg: Cfg, results: list[dict]) -> np.ndarray:
    out = np.empty((cfg.TOK, cfg.M), np.float32)
    for core in range(cfg.n_cores):
        shard = np.asarray(results[core]["out_rs"])  # (TOK//n_cores, M)
        cum = 0
        for toff, sz in RS_CHUNKS_HOST:
            sh = sz // cfg.n_cores
            out[toff + core * sh:toff + (core + 1) * sh, :] = \
                shard[cum:cum + sh, :]
            cum += sh
    return out.reshape(cfg.B, cfg.L, cfg.M).astype(np.float32)


def kernel(**inputs) -> np.ndarray:
    cfg = FULL
    from concourse.bass_utils import run_bass_kernel_spmd
    nc = build_program(cfg)
    in_maps = host_prep(cfg, inputs)
    res = run_bass_kernel_spmd(nc, in_maps, core_ids=list(range(cfg.n_cores)))
    return gather_out(cfg, res.results)


# revision 83
# speedup vs baseline: 1.1061x; 1.1061x over previous
"""Bidirectional Mamba block (in_proj -> depthwise causal conv -> SiLU ->
forward+backward S6 selective scan -> gated combine -> out_proj) as a
Trainium2 Bass/Tile SPMD kernel over 8 NeuronCores.

Sharding: tensor-parallel over d_inner (256 channels per core). The conv and
the S6 scans are channel-independent, so they need no communication.

Collectives are pipelined per batch element:
  * AllReduce(b) of both directions' partial x-projections dbc = u @ Wx^T
    ([192, 1024] fp32) is issued as soon as batch b's conv is done; AR(b0)
    overlaps the b1 front-end and the res projection, AR(b1) overlaps the
    b0 scan phase.
  * The partial out-projection is cast to bf16 and ReduceScattered in 4
    chunks (2 per batch); the b0 chunks overlap the b1 scan phase.  (More
    or uneven chunks measured slower: each extra collective costs tens of
    microseconds of serialized launch/ring time.)

Scan layout: partitions = (8 channels x 16 states), free dim = L, one scan
instruction per (direction, batch, channel-group) on the DVE hardware scan
(fp32 internal state; the per-element feedback bubble pins it at ~2.2
cycles/element regardless of dtype).  The x transposes, in_proj and the
whole scan phase run in bf16 (rel err ~6e-3 vs the 2e-2 budget).

The entire dBu -> scan -> hC chain runs on the vector engine: dBu multiplies
the PE-produced w replica straight out of PSUM (1x rate), hC runs at the 2x
bf16 rate.  Offloading dBu/hC to GpSimd measures *slower* -- a GpSimd
tensor op co-running with a scan inflates the scan ~1.8us via SBUF port
contention, more than the offload saves.

PE array tiling: the delta/w replication matmuls have a true contraction of
8 rows, so four channel groups run concurrently in 32x128 row-tiles
(tile_position=(32i, 0), 32-row stationaries selecting 8 rows); the
per-group state reduction y = sum_n C*h has 8 output rows, so four groups
run concurrently in 128x32 column tiles (tile_position=(0, 32i)), and the
skip term u*(fD+bD) is folded into the same PSUM accumulation as a fifth,
diagonal column-tiled matmul.  Channel group g = 4i+k lives at partition
base 32i+8k, which places every concurrent quad {k, 4+k, 8+k, 12+k} in four
distinct array quadrants without any host-side channel permutation.
"""

import os
import sys

for _p in ("/opt/trn_rl_repo", "/root/.axon_site/_ro/trn_rl_repo"):
    if os.path.isdir(_p) and _p not in sys.path:
        sys.path.append(_p)

from dataclasses import dataclass

import ml_dtypes
import numpy as np

import concourse.bass as bass
import concourse.mybir as mybir
import concourse.tile as tile
from concourse import bacc

DT = mybir.dt.float32
F32R = mybir.dt.float32r
BF = mybir.dt.bfloat16
AF = mybir.ActivationFunctionType
OP = mybir.AluOpType


@dataclass(frozen=True)
class Cfg:
    n_cores: int = 8
    B: int = 2
    L: int = 1024
    M: int = 1024      # d_model
    DI: int = 2048     # d_inner
    N: int = 16        # d_state
    R: int = 64        # dt_rank
    KC: int = 4        # conv kernel

    @property
    def DC(self):  # channels per core
        return self.DI // self.n_cores

    @property
    def TOK(self):
        return self.B * self.L

    @property
    def P_CH(self):  # partitions per channel tile
        return min(128, self.DC)

    @property
    def CHT(self):  # channel tiles per core
        return self.DC // self.P_CH

    @property
    def NT(self):  # scan tiles per (dir, batch): 8 channels each
        return self.DC // 8

    @property
    def TPC(self):  # scan tiles per channel tile
        return self.P_CH // 8

    @property
    def FCH(self):  # matmul moving-dim chunk over tokens
        return min(512, self.L)

    @property
    def E(self):
        return self.R + 2 * self.N

    def check(self):
        assert self.DC % 8 == 0 and self.DC % self.P_CH == 0
        assert self.M % 128 == 0
        assert self.TOK % 128 == 0 and self.TOK % self.FCH == 0
        assert self.L % min(512, self.L) == 0
        assert self.N == 16 and self.TPC == 16


FULL = Cfg()


def build_consts(cfg: Cfg):
    """Selection matrices used as PE 'weights' (exact 0/1 values)."""
    P = 128
    ident = np.eye(P, dtype=np.float32)
    # rall32[32i+r, k, col] = 1 iff r in [8k, 8k+8) and col//16 == r-8k:
    # a 32-row stationary (32-aligned, as the PE tiling requires) that
    # replicates the 8 channel rows of group 4i+k into 128 (ch x state)
    # rows; the other 24 rows of the quadrant contribute zeros.
    rall32 = np.zeros((P, 4, P), np.float32)
    for p in range(P):
        r = p % 32
        for k in range(4):
            if 8 * k <= r < 8 * k + 8:
                rall32[p, k, (r - 8 * k) * 16:(r - 8 * k + 1) * 16] = 1.0
    # tsel[:, which, :]: out[p] = src[16*which + p%16]  (B/C replication)
    t_sel = np.zeros((2 * cfg.N, 2, P), np.float32)
    for which in range(2):
        for p in range(P):
            t_sel[cfg.N * which + p % 16, which, p] = 1.0
    # sall32[p, k, j] = 1 iff j == 8k + p//16: reduce the 16 state rows of
    # channel e into output row 8k+e of a 32-row column tile.
    sall32 = np.zeros((P, 4, 32), np.float32)
    for p in range(P):
        for k in range(4):
            sall32[p, k, 8 * k + p // 16] = 1.0
    # ddiag[p, c, i, j] = dsum[c*128+p] iff p == 32i+j: adds u*(fD+bD) into
    # the y accumulation as a 4-way column-tiled diagonal matmul.
    return ident, rall32, t_sel, sall32


def build_ddiag(cfg: Cfg, dsum_local: np.ndarray):
    P = 128
    ddiag = np.zeros((P, cfg.CHT, 4, 32), np.float32)
    for c in range(cfg.CHT):
        for p in range(P):
            i, j = p // 32, p % 32
            ddiag[p, c, i, j] = dsum_local[c * P + p]
    return ddiag.reshape(P, cfg.CHT * 4 * 32)


def build_program(cfg: Cfg) -> bass.Bass:
    cfg.check()
    P = 128
    TOK, L, M = cfg.TOK, cfg.L, cfg.M
    DC, CHT, P_CH = cfg.DC, cfg.CHT, cfg.P_CH
    FCH = cfg.FCH
    MT = M // P               # m tiles
    E, R, N = cfg.E, cfg.R, cfg.N
    LCH = L // FCH            # matmul chunks per batch
    TBB = L // P              # token blocks per batch

    nc = bacc.Bacc(
        "TRN2", target_bir_lowering=False, debug=False, num_devices=cfg.n_cores
    )

    # ---- kernel I/O ----
    x_d = nc.dram_tensor("x", [TOK, M], DT, kind="ExternalInput")
    winuT_d = nc.dram_tensor("winuT", [M, DC], BF, kind="ExternalInput")
    winrT_d = nc.dram_tensor("winrT", [M, DC], BF, kind="ExternalInput")
    wconv_d = nc.dram_tensor("wconv", [P, CHT * cfg.KC], DT, kind="ExternalInput")
    bconv_d = nc.dram_tensor("bconv", [P, CHT], DT, kind="ExternalInput")
    wxT_d = {d: nc.dram_tensor(f"wx{d}T", [DC, E], F32R, kind="ExternalInput")
             for d in "fb"}
    wdtT_d = {d: nc.dram_tensor(f"wdt{d}T", [R, DC], F32R, kind="ExternalInput")
              for d in "fb"}
    bdt_d = {d: nc.dram_tensor(f"bdt{d}", [P, CHT], DT, kind="ExternalInput")
             for d in "fb"}
    acol_d = {d: nc.dram_tensor(f"acol{d}", [P, CHT * cfg.TPC], DT,
                                kind="ExternalInput")
              for d in "fb"}
    woutT_d = nc.dram_tensor("woutT", [DC, M], BF, kind="ExternalInput")
    ident_d = nc.dram_tensor("ident", [P, P], DT, kind="ExternalInput")
    rall32_d = nc.dram_tensor("rall32", [P, 4 * P], BF, kind="ExternalInput")
    tsel_d = nc.dram_tensor("tsel", [2 * N, 2 * P], F32R, kind="ExternalInput")
    sall32_d = nc.dram_tensor("sall32", [P, 4 * 32], BF, kind="ExternalInput")
    ddiag_d = nc.dram_tensor("ddiag", [P, CHT * 4 * 32], BF,
                             kind="ExternalInput")

    out_d = nc.dram_tensor("out_rs", [TOK // cfg.n_cores, M], BF,
                           kind="ExternalOutput")

    rg = [list(range(cfg.n_cores))]
    cc_space = "Shared" if cfg.n_cores > 4 else "Local"
    # ReduceScatter chunk schedule (token offset, size): the final chunks are
    # smaller so the exposed tail after the last y is short.
    RS_CHUNKS = [(0, 512), (512, 512), (1024, 1024)]
    RS_OF_B = {0: [0, 1], 1: [2]}

    with tile.TileContext(nc) as tc:
        with tc.tile_pool(name="persist", bufs=1) as pp, \
             tc.tile_pool(name="dram", bufs=1, space="DRAM") as dp:

            # ---------- persistent SBUF (small weights) --------------------
            ident_s = pp.tile([P, P], DT)
            nc.sync.dma_start(ident_s[:], ident_d.ap())
            identb_s = pp.tile([P, P], BF)
            nc.scalar.copy(identb_s[:], ident_s[:])
            rall32_s = pp.tile([P, 4, P], BF)
            nc.sync.dma_start(rall32_s[:], rall32_d.ap().rearrange(
                "p (a b) -> p a b", a=4))
            tsel_s = pp.tile([2 * N, 2, P], F32R)
            nc.sync.dma_start(tsel_s[:], tsel_d.ap().rearrange(
                "k (a b) -> k a b", a=2))
            sall32_s = pp.tile([P, 4, 32], BF)
            nc.sync.dma_start(sall32_s[:], sall32_d.ap().rearrange(
                "p (a b) -> p a b", a=4))
            ddiag_s = pp.tile([P, CHT, 4, 32], BF)
            nc.sync.dma_start(ddiag_s[:], ddiag_d.ap().rearrange(
                "p (c a b) -> p c a b", c=CHT, a=4))
            wconv_s = pp.tile([P, CHT, cfg.KC], DT)
            nc.sync.dma_start(wconv_s[:], wconv_d.ap().rearrange(
                "p (c k) -> p c k", c=CHT))
            bconv_s = pp.tile([P, CHT], DT)
            nc.sync.dma_start(bconv_s[:], bconv_d.ap())
            wx_s, wdt_s, bdt_s, acol_s = {}, {}, {}, {}
            for d in "fb":
                wx_s[d] = pp.tile([P_CH, CHT, E], F32R, name=f"wx{d}_s")
                nc.sync.dma_start(wx_s[d][:], wxT_d[d].ap().rearrange(
                    "(c p) e -> p c e", p=P_CH))
                wdt_s[d] = pp.tile([R, DC], F32R, name=f"wdt{d}_s")
                nc.sync.dma_start(wdt_s[d][:], wdtT_d[d].ap())
                bdt_s[d] = pp.tile([P, CHT], DT, name=f"bdt{d}_s")
                nc.sync.dma_start(bdt_s[d][:], bdt_d[d].ap())
                acol_s[d] = pp.tile([P, CHT * cfg.TPC], DT, name=f"acol{d}_s")
                nc.sync.dma_start(acol_s[d][:], acol_d[d].ap())
            wout_s = pp.tile([P_CH, CHT, M], BF)
            nc.sync.dma_start(wout_s[:], woutT_d.ap().rearrange(
                "(c p) m -> p c m", p=P_CH))

            u_c = [pp.tile([P_CH, TOK], F32R, name=f"u_c{c}") for c in range(CHT)]
            u_bf = [pp.tile([P_CH, TOK], BF, name=f"u_bf{c}") for c in range(CHT)]
            sres = [pp.tile([P_CH, TOK], BF, name=f"sres{c}")
                    for c in range(CHT)]
            y_f = [pp.tile([P_CH, TOK], BF, name=f"y_f{c}") for c in range(CHT)]

            # per-batch dbc partials/reductions (both directions merged)
            dbc_part = [dp.tile([2 * E, L], DT, name=f"dbc_part{b}")
                        for b in range(cfg.B)]
            dbc_red = [dp.tile([2 * E, L], DT, addr_space=cc_space,
                               name=f"dbc_red{b}") for b in range(cfg.B)]
            out_part = [dp.tile([sz, M], BF, name=f"out_part{r}")
                        for r, (_, sz) in enumerate(RS_CHUNKS)]
            out_rs = [dp.tile([sz // cfg.n_cores, M], BF, name=f"out_rs{r}")
                      for r, (_, sz) in enumerate(RS_CHUNKS)]

            # ================= front-end (per batch) + res-proj ============
            with tc.tile_pool(name="proj", bufs=1) as jp, \
                 tc.tile_pool(name="proj_ps", bufs=1, space="PSUM") as jpp:
                xT = [jp.tile([P, TOK], BF, name=f"xT{mt}") for mt in range(MT)]
                win_s = jp.tile([P, MT, 2 * DC], BF)
                nc.sync.dma_start(win_s[:, :, :DC], winuT_d.ap().rearrange(
                    "(a p) c -> p a c", p=P))
                nc.sync.dma_start(win_s[:, :, DC:], winrT_d.ap().rearrange(
                    "(a p) c -> p a c", p=P))

                upad = [[jp.tile([P_CH, cfg.KC - 1 + L], DT,
                                 name=f"upad{c}_{b}")
                         for b in range(cfg.B)] for c in range(CHT)]

                TPG = min(4, MT)
                for b in range(cfg.B):
                    bo = b * L
                    for half in range(LCH):
                        t0 = b * TBB + half * (FCH // P)
                        for tb in range(t0, t0 + FCH // P):
                            xsb = jp.tile([P, M], DT, tag="xsb", bufs=2,
                                          name="xsb")
                            nc.sync.dma_start(
                                xsb[:], x_d.ap()[tb * P:(tb + 1) * P, :])
                            xbb = jp.tile([P, M], BF, tag="xbb", bufs=2,
                                          name="xbb")
                            nc.scalar.copy(xbb[:], xsb[:])
                            for mg in range(MT // TPG):
                                tp_ps = jpp.tile([P, TPG * P], BF, tag="tp",
                                                 bufs=4, name="tp_ps")
                                for k in range(TPG):
                                    mt = mg * TPG + k
                                    nc.tensor.transpose(
                                        tp_ps[:, k * P:(k + 1) * P],
                                        xbb[:, mt * P:(mt + 1) * P],
                                        identb_s[:])
                                for k in range(TPG):
                                    mt = mg * TPG + k
                                    nc.vector.tensor_copy(
                                        xT[mt][:, tb * P:(tb + 1) * P],
                                        tp_ps[:, k * P:(k + 1) * P])
                        f0 = half * FCH
                        for c in range(CHT):
                            ups = jpp.tile([P_CH, FCH], DT, tag="mm", bufs=4,
                                           name="ups")
                            for kt in range(MT):
                                nc.tensor.matmul(
                                    ups[:],
                                    win_s[:, kt, c * P_CH:(c + 1) * P_CH],
                                    xT[kt][:, bo + f0:bo + f0 + FCH],
                                    start=(kt == 0), stop=(kt == MT - 1))
                            nc.scalar.copy(
                                upad[c][b][:, cfg.KC - 1 + f0:
                                           cfg.KC - 1 + f0 + FCH], ups[:])

                    for c in range(CHT):
                        nc.gpsimd.memset(upad[c][b][:, :cfg.KC - 1], 0.0)
                        # depthwise causal conv + SiLU
                        acc = None
                        for k in range(cfg.KC):
                            nxt = jp.tile([P_CH, L], DT, tag="cacc",
                                          bufs=2, name="cacc")
                            tap = upad[c][b][:, k:k + L]
                            wk = wconv_s[:P_CH, c, k:k + 1]
                            if acc is None:
                                nc.vector.tensor_scalar(
                                    nxt[:], tap, wk,
                                    bconv_s[:P_CH, c:c + 1],
                                    OP.mult, OP.add)
                            else:
                                nc.vector.scalar_tensor_tensor(
                                    nxt[:], tap, wk, acc[:],
                                    OP.mult, OP.add)
                            acc = nxt
                        sg2 = jp.tile([P_CH, L], DT, tag="sg2", bufs=2,
                                      name="sg2")
                        nc.scalar.activation(sg2[:], acc[:], AF.Sigmoid)
                        nc.gpsimd.tensor_tensor(
                            u_c[c][:, bo:bo + L], acc[:], sg2[:], OP.mult)
                        nc.scalar.copy(
                            u_bf[c][:, bo:bo + L],
                            u_c[c][:, bo:bo + L].bitcast(DT))

                    # dbc partials (both dirs) for this batch -> AllReduce(b)
                    for di, d in enumerate("fb"):
                        for lh in range(LCH):
                            f0 = lh * FCH
                            bps = jpp.tile([E, FCH], DT, tag="mm", bufs=4,
                                           name="bps")
                            for c in range(CHT):
                                nc.tensor.matmul(
                                    bps[:],
                                    wx_s[d][:, c, :],
                                    u_c[c][:, bo + f0:bo + f0 + FCH],
                                    start=(c == 0), stop=(c == CHT - 1))
                            bst = jp.tile([E, FCH], DT, tag="bst", bufs=3,
                                          name="bst")
                            nc.scalar.copy(bst[:], bps[:])
                            nc.sync.dma_start(
                                dbc_part[b][di * E:(di + 1) * E,
                                            f0:f0 + FCH], bst[:])
                    nc.gpsimd.collective_compute(
                        "AllReduce", OP.add, replica_groups=rg,
                        ins=[dbc_part[b].opt()], outs=[dbc_red[b].opt()])

                # res projection (overlaps the AllReduces)
                for c in range(CHT):
                    for fc in range(TOK // FCH):
                        f0 = fc * FCH
                        rps = jpp.tile([P_CH, FCH], DT, tag="mm", bufs=4,
                                       name="rps")
                        for kt in range(MT):
                            nc.tensor.matmul(
                                rps[:],
                                win_s[:, kt, DC + c * P_CH:DC + (c + 1) * P_CH],
                                xT[kt][:, f0:f0 + FCH],
                                start=(kt == 0), stop=(kt == MT - 1))
                        sg = jp.tile([P_CH, FCH], BF, tag="sg", bufs=2,
                                     name="sg")
                        nc.scalar.activation(sg[:], rps[:], AF.Sigmoid)
                        nc.vector.tensor_tensor(sres[c][:, f0:f0 + FCH],
                                                rps[:], sg[:], OP.mult)

            # ================= scan phase + out_proj (per batch) ===========
            with tc.tile_pool(name="scan_sb", bufs=1) as sp, \
                 tc.tile_pool(name="scan_ps", bufs=1, space="PSUM") as spp, \
                 tc.tile_pool(name="comb", bufs=1) as kp:

                def rep_tile():
                    return spp.tile([P, 2 * FCH], DT, tag="rep", bufs=3,
                                    name="rep")

                for b in range(cfg.B):
                    bo = b * L
                    for di, d in enumerate("fb"):
                        off = di * E
                        dt_sb = sp.tile([R, L], F32R, tag="dt", bufs=2,
                                        name=f"dt_{d}{b}")
                        nc.sync.dma_start(
                            dt_sb[:], dbc_red[b][off:off + R, :].bitcast(F32R))
                        bc_sb = sp.tile([2 * N, L], F32R, tag="bc", bufs=2,
                                        name=f"bc_{d}{b}")
                        nc.sync.dma_start(
                            bc_sb[:],
                            dbc_red[b][off + R:off + E, :].bitcast(F32R))

                        # B/C replicated across the 8-channel groups
                        brep = sp.tile([P, L], BF, tag="brep", bufs=2,
                                       name=f"brep{d}")
                        crep = sp.tile([P, L], BF, tag="crep", bufs=2,
                                       name=f"crep{d}")
                        for which, rep in ((0, brep), (1, crep)):
                            ps = rep_tile()
                            for lh in range(LCH):
                                o = lh * FCH
                                nc.tensor.matmul(
                                    ps[:, o:o + FCH],
                                    tsel_s[:, which, :],
                                    bc_sb[:, o:o + FCH],
                                    start=True, stop=True)
                            nc.scalar.copy(rep[:], ps[:])

                        # delta = softplus(dt @ WdtT + bdt); w = delta * u
                        delta = [sp.tile([P_CH, L], BF, tag=f"delta{c}",
                                         bufs=2, name=f"delta_{d}{c}")
                                 for c in range(CHT)]
                        w_s = [sp.tile([P_CH, L], BF, tag=f"w{c}", bufs=2,
                                       name=f"w_{d}{c}") for c in range(CHT)]
                        for c in range(CHT):
                            ps = rep_tile()
                            for lh in range(LCH):
                                o = lh * FCH
                                nc.tensor.matmul(
                                    ps[:, o:o + FCH],
                                    wdt_s[d][:, c * P_CH:(c + 1) * P_CH],
                                    dt_sb[:, o:o + FCH],
                                    start=True, stop=True)
                            spt = sp.tile([P_CH, L], DT, tag="spt", bufs=2,
                                          name="spt")
                            # softplus(x + bdt) = ln(1 + exp(x + bdt))
                            nc.scalar.activation(
                                spt[:], ps[:], AF.Exp,
                                bias=bdt_s[d][:P_CH, c:c + 1])
                            nc.scalar.activation(
                                delta[c][:], spt[:], AF.Ln, bias=1.0)
                            nc.vector.tensor_tensor(
                                w_s[c][:], delta[c][:],
                                u_bf[c][:, bo:bo + L], OP.mult)

                        for c in range(CHT):
                            hC = [None] * 16
                            for k in range(4):      # concurrent quad sets
                                dA, dBu = {}, {}
                                dpq, wpq = {}, {}
                                for i in range(4):
                                    g = 4 * i + k
                                    dpq[g] = rep_tile()
                                    for lh in range(LCH):
                                        q = lh * FCH
                                        nc.tensor.matmul(
                                            dpq[g][:, q:q + FCH],
                                            rall32_s[32 * i:32 * i + 32,
                                                     k, :],
                                            delta[c][32 * i:32 * i + 32,
                                                     q:q + FCH],
                                            start=True, stop=True,
                                            tile_position=(32 * i, 0))
                                for i in range(4):
                                    g = 4 * i + k
                                    j = c * 16 + g
                                    dA[g] = sp.tile([P, L], BF, tag="dA",
                                                    bufs=6, name="dA")
                                    nc.scalar.activation(
                                        dA[g][:], dpq[g][:], AF.Exp,
                                        scale=acol_s[d][:, j:j + 1])
                                for i in range(4):
                                    g = 4 * i + k
                                    wpq[g] = rep_tile()
                                    for lh in range(LCH):
                                        q = lh * FCH
                                        nc.tensor.matmul(
                                            wpq[g][:, q:q + FCH],
                                            rall32_s[32 * i:32 * i + 32,
                                                     k, :],
                                            w_s[c][32 * i:32 * i + 32,
                                                   q:q + FCH],
                                            start=True, stop=True,
                                            tile_position=(32 * i, 0))
                                for i in range(4):
                                    g = 4 * i + k
                                    dBu[g] = sp.tile([P, L], BF, tag="dBu",
                                                     bufs=6, name="dBu")
                                    if i == 0:
                                        nc.vector.tensor_tensor(
                                            dBu[g][:], wpq[g][:], brep[:],
                                            OP.mult)
                                    else:
                                        wsb = sp.tile([P, L], BF, tag="wsb",
                                                      bufs=3, name="wsb")
                                        nc.scalar.copy(wsb[:], wpq[g][:])
                                        nc.vector.tensor_tensor(
                                            dBu[g][:], wsb[:], brep[:],
                                            OP.mult)
                                for i in range(4):
                                    g = 4 * i + k
                                    h = sp.tile([P, L], BF, tag="h", bufs=10,
                                                name="h")
                                    if d == "f":
                                        nc.vector.tensor_tensor_scan(
                                            h[:], dA[g][:], dBu[g][:], 0.0,
                                            OP.mult, OP.add)
                                    else:
                                        nc.vector.tensor_tensor_scan(
                                            h[:, ::-1], dA[g][:, ::-1],
                                            dBu[g][:, ::-1],
                                            0.0, OP.mult, OP.add)
                                    hC[g] = sp.tile([P, L], BF, tag="hC",
                                                    bufs=17, name="hC")
                                    nc.vector.tensor_tensor(
                                        hC[g][:], h[:], crep[:], OP.mult)
                            # column-tiled y reduction over all 16 groups
                            y_ps = spp.tile([P_CH, L], DT, tag="y", bufs=1,
                                            name="y_ps")
                            for lh in range(LCH):
                                q = lh * FCH
                                for k in range(4):
                                    for i in range(4):
                                        g = 4 * i + k
                                        nc.tensor.matmul(
                                            y_ps[32 * i:32 * i + 32,
                                                 q:q + FCH],
                                            sall32_s[:, k, :],
                                            hC[g][:, q:q + FCH],
                                            start=(k == 0),
                                            stop=(k == 3 and d == "f"),
                                            tile_position=(0, 32 * i))
                                if d == "b":
                                    # add u*(fD+bD) into the accumulation
                                    for i in range(4):
                                        nc.tensor.matmul(
                                            y_ps[32 * i:32 * i + 32,
                                                 q:q + FCH],
                                            ddiag_s[:, c, i, :],
                                            u_bf[c][:, bo + q:bo + q + FCH],
                                            start=False, stop=True,
                                            tile_position=(0, 32 * i))
                            if d == "f":
                                nc.scalar.copy(y_f[c][:, bo:bo + L], y_ps[:])
                            else:
                                # fused combine:
                                # y = (y_f + y_b + u*(fD+bD)) * (0.5*silu(res))
                                # (the 0.5 is folded into W_out host-side)
                                ysl = y_f[c][:, bo:bo + L]
                                t1 = kp.tile([P_CH, L], BF, tag="t5", bufs=2,
                                             name="t1")
                                nc.vector.tensor_tensor(t1[:], y_ps[:],
                                                        ysl, OP.add)
                                nc.vector.tensor_tensor(
                                    ysl, t1[:], sres[c][:, bo:bo + L],
                                    OP.mult)

                    # out_proj + ReduceScatter for this batch
                    MFC = min(512, M)
                    for r in RS_OF_B[b]:
                        toff, sz = RS_CHUNKS[r]
                        for tr in range(sz // P):
                            t0 = toff + tr * P
                            ops = spp.tile([P, M], DT,
                                           tag=("rep" if b == 1 else "y"),
                                           bufs=(3 if b == 1 else 1),
                                           name="ops")
                            for mc in range(M // MFC):
                                o = mc * MFC
                                for c in range(CHT):
                                    nc.tensor.matmul(
                                        ops[:, o:o + MFC],
                                        y_f[c][:, t0:t0 + P],
                                        wout_s[:, c, o:o + MFC],
                                        start=(c == 0), stop=(c == CHT - 1))
                            ost = kp.tile([P, M], BF, tag="ost", bufs=3,
                                          name="ost")
                            nc.scalar.copy(ost[:], ops[:])
                            nc.sync.dma_start(
                                out_part[r][tr * P:(tr + 1) * P, :], ost[:])
                        nc.gpsimd.collective_compute(
                            "ReduceScatter", OP.add, replica_groups=rg,
                            ins=[out_part[r].opt()], outs=[out_rs[r].opt()])
                        nc.sync.dma_start(
                            out_d.ap()[toff // cfg.n_cores:
                                       (toff + sz) // cfg.n_cores, :],
                            out_rs[r][:])

    nc.compile()
    return nc


# --------------------------------------------------------------------------
# host side
# --------------------------------------------------------------------------

def host_prep(cfg: Cfg, inputs: dict) -> list[dict]:
    """Slice the full-model inputs into one input map per core."""
    P = 128
    f32 = np.float32

    def g(name):
        return np.asarray(inputs[name], f32)

    x = g("x").reshape(cfg.TOK, cfg.M)
    W_in = g("W_in")
    W_conv = g("W_conv").reshape(cfg.DI, cfg.KC)
    b_conv = g("b_conv")
    W_out = g("W_out")
    ident, rall32, t_sel, sall32 = build_consts(cfg)
    tsel_flat = t_sel.reshape(2 * cfg.N, 2 * P)
    sall32_flat = sall32.reshape(P, 4 * 32)
    rall32_flat = rall32.reshape(P, 4 * P)

    per = {}
    for d in "fb":
        per[d] = dict(
            A=-np.exp(g(d + "A_log")),            # (DI, N)
            D=g(d + "D"),
            Wx=g(d + "Wx"),                       # (E, DI)
            Wdt=g(d + "Wdt"),                     # (DI, R)
            bdt=g(d + "bdt"),
        )

    def col_layout(v):  # (DC,) -> (P_CH, CHT): [p, c] = v[c*P_CH + p]
        return np.ascontiguousarray(
            v.reshape(cfg.CHT, cfg.P_CH).T.astype(f32))

    def pad_p(a):  # pad partition dim up to 128
        if a.shape[0] == P:
            return np.ascontiguousarray(a.astype(f32))
        out = np.zeros((P,) + a.shape[1:], f32)
        out[:a.shape[0]] = a
        return out

    in_maps = []
    for core in range(cfg.n_cores):
        c0 = core * cfg.DC
        ch = slice(c0, c0 + cfg.DC)
        m = {
            "x": x,
            "winuT": np.ascontiguousarray(
                W_in[ch, :].T.astype(ml_dtypes.bfloat16)),
            "winrT": np.ascontiguousarray(
                W_in[cfg.DI + c0:cfg.DI + c0 + cfg.DC, :]
                .T.astype(ml_dtypes.bfloat16)),
            "wconv": pad_p(
                W_conv[ch].reshape(cfg.CHT, cfg.P_CH, cfg.KC)
                .transpose(1, 0, 2).reshape(cfg.P_CH, cfg.CHT * cfg.KC)),
            "bconv": pad_p(col_layout(b_conv[ch])),
            "woutT": np.ascontiguousarray(
                (W_out[:, ch].T * 0.5).astype(ml_dtypes.bfloat16)),
            "ident": ident,
            "ddiag": build_ddiag(
                cfg, (per["f"]["D"][ch] + per["b"]["D"][ch]).astype(f32)
            ).astype(ml_dtypes.bfloat16),
            "rall32": rall32_flat.astype(ml_dtypes.bfloat16),
            "tsel": tsel_flat,
            "sall32": sall32_flat.astype(ml_dtypes.bfloat16),
        }
        for d in "fb":
            pd = per[d]
            m[f"wx{d}T"] = np.ascontiguousarray(pd["Wx"][:, ch].T)
            m[f"wdt{d}T"] = np.ascontiguousarray(pd["Wdt"][ch, :].T)
            m[f"bdt{d}"] = pad_p(col_layout(pd["bdt"][ch]))
            # A columns: [p, j] = A[8j + p//16, p%16] (local channels)
            Ac = pd["A"][ch]                       # (DC, N)
            acol = np.empty((P, cfg.NT), f32)
            pidx = np.arange(P)
            for j in range(cfg.NT):
                acol[:, j] = Ac[8 * j + pidx // 16, pidx % 16]
            m[f"acol{d}"] = acol
        in_maps.append({k: np.ascontiguousarray(v) for k, v in m.items()})
    return in_maps


RS_CHUNKS_HOST = [(0, 512), (512, 512), (1024, 1024)]


def gather_out(cfg: Cfg, results: list[dict]) -> np.ndarray:
    out = np.empty((cfg.TOK, cfg.M), np.float32)
    for core in range(cfg.n_cores):
        shard = np.asarray(results[core]["out_rs"])  # (TOK//n_cores, M)
        cum = 0
        for toff, sz in RS_CHUNKS_HOST:
            sh = sz // cfg.n_cores
            out[toff + core * sh:toff + (core + 1) * sh, :] = \
                shard[cum:cum + sh, :]
            cum += sh
    return out.reshape(cfg.B, cfg.L, cfg.M).astype(np.float32)


def kernel(**inputs) -> np.ndarray:
    cfg = FULL
    from concourse.bass_utils import run_bass_kernel_spmd
    nc = build_program(cfg)
    in_maps = host_prep(cfg, inputs)
    res = run_bass_kernel_spmd(nc, in_maps, core_ids=list(range(cfg.n_cores)))
    return gather_out(cfg, res.results)


# revision 87
# speedup vs baseline: 1.1491x; 1.0389x over previous
"""Bidirectional Mamba block (in_proj -> depthwise causal conv -> SiLU ->
forward+backward S6 selective scan -> gated combine -> out_proj) as a
Trainium2 Bass/Tile SPMD kernel over 8 NeuronCores.

Sharding: tensor-parallel over d_inner (256 channels per core). The conv and
the S6 scans are channel-independent, so they need no communication.

Collectives are pipelined per batch element:
  * AllReduce(b) of both directions' partial x-projections dbc = u @ Wx^T
    ([192, 1024] fp32) is issued as soon as batch b's conv is done; AR(b0)
    overlaps the b1 front-end and the res projection, AR(b1) overlaps the
    b0 scan phase.
  * The partial out-projection is cast to bf16 and ReduceScattered in 4
    chunks (2 per batch); the b0 chunks overlap the b1 scan phase.  (More
    or uneven chunks measured slower: each extra collective costs tens of
    microseconds of serialized launch/ring time.)

Scan layout: partitions = (8 channels x 16 states), free dim = L, one scan
instruction per (direction, batch, channel-group) on the DVE hardware scan
(fp32 internal state; the per-element feedback bubble pins it at ~2.2
cycles/element regardless of dtype).  The x transposes, in_proj and the
whole scan phase run in bf16 (rel err ~6e-3 vs the 2e-2 budget).

The entire dBu -> scan -> hC chain runs on the vector engine: dBu multiplies
the PE-produced w replica straight out of PSUM (1x rate), hC runs at the 2x
bf16 rate.  Offloading dBu/hC to GpSimd measures *slower* -- a GpSimd
tensor op co-running with a scan inflates the scan ~1.8us via SBUF port
contention, more than the offload saves.

PE array tiling: the delta/w replication matmuls have a true contraction of
8 rows, so four channel groups run concurrently in 32x128 row-tiles
(tile_position=(32i, 0), 32-row stationaries selecting 8 rows); the
per-group state reduction y = sum_n C*h has 8 output rows, so four groups
run concurrently in 128x32 column tiles (tile_position=(0, 32i)), and the
skip term u*(fD+bD) is folded into the same PSUM accumulation as a fifth,
diagonal column-tiled matmul.  Channel group g = 4i+k lives at partition
base 32i+8k, which places every concurrent quad {k, 4+k, 8+k, 12+k} in four
distinct array quadrants without any host-side channel permutation.
"""

import os
import sys

for _p in ("/opt/trn_rl_repo", "/root/.axon_site/_ro/trn_rl_repo"):
    if os.path.isdir(_p) and _p not in sys.path:
        sys.path.append(_p)

from dataclasses import dataclass

import ml_dtypes
import numpy as np

import concourse.bass as bass
import concourse.mybir as mybir
import concourse.tile as tile
from concourse import bacc

DT = mybir.dt.float32
F32R = mybir.dt.float32r
BF = mybir.dt.bfloat16
AF = mybir.ActivationFunctionType
OP = mybir.AluOpType


@dataclass(frozen=True)
class Cfg:
    n_cores: int = 8
    B: int = 2
    L: int = 1024
    M: int = 1024      # d_model
    DI: int = 2048     # d_inner
    N: int = 16        # d_state
    R: int = 64        # dt_rank
    KC: int = 4        # conv kernel

    @property
    def DC(self):  # channels per core
        return self.DI // self.n_cores

    @property
    def TOK(self):
        return self.B * self.L

    @property
    def P_CH(self):  # partitions per channel tile
        return min(128, self.DC)

    @property
    def CHT(self):  # channel tiles per core
        return self.DC // self.P_CH

    @property
    def NT(self):  # scan tiles per (dir, batch): 8 channels each
        return self.DC // 8

    @property
    def TPC(self):  # scan tiles per channel tile
        return self.P_CH // 8

    @property
    def FCH(self):  # matmul moving-dim chunk over tokens
        return min(512, self.L)

    @property
    def E(self):
        return self.R + 2 * self.N

    def check(self):
        assert self.DC % 8 == 0 and self.DC % self.P_CH == 0
        assert self.M % 128 == 0
        assert self.TOK % 128 == 0 and self.TOK % self.FCH == 0
        assert self.L % min(512, self.L) == 0
        assert self.N == 16 and self.TPC == 16


FULL = Cfg()


def build_consts(cfg: Cfg):
    """Selection matrices used as PE 'weights' (exact 0/1 values)."""
    P = 128
    ident = np.eye(P, dtype=np.float32)
    # rall32[32i+r, k, col] = 1 iff r in [8k, 8k+8) and col//16 == r-8k:
    # a 32-row stationary (32-aligned, as the PE tiling requires) that
    # replicates the 8 channel rows of group 4i+k into 128 (ch x state)
    # rows; the other 24 rows of the quadrant contribute zeros.
    rall32 = np.zeros((P, 4, P), np.float32)
    for p in range(P):
        r = p % 32
        for k in range(4):
            if 8 * k <= r < 8 * k + 8:
                rall32[p, k, (r - 8 * k) * 16:(r - 8 * k + 1) * 16] = 1.0
    # tsel[:, which, :]: out[p] = src[16*which + p%16]  (B/C replication)
    t_sel = np.zeros((2 * cfg.N, 2, P), np.float32)
    for which in range(2):
        for p in range(P):
            t_sel[cfg.N * which + p % 16, which, p] = 1.0
    # sall32[p, k, j] = 1 iff j == 8k + p//16: reduce the 16 state rows of
    # channel e into output row 8k+e of a 32-row column tile.
    sall32 = np.zeros((P, 4, 32), np.float32)
    for p in range(P):
        for k in range(4):
            sall32[p, k, 8 * k + p // 16] = 1.0
    # ddiag[p, c, i, j] = dsum[c*128+p] iff p == 32i+j: adds u*(fD+bD) into
    # the y accumulation as a 4-way column-tiled diagonal matmul.
    return ident, rall32, t_sel, sall32


def build_ddiag(cfg: Cfg, dsum_local: np.ndarray):
    P = 128
    ddiag = np.zeros((P, cfg.CHT, 4, 32), np.float32)
    for c in range(cfg.CHT):
        for p in range(P):
            i, j = p // 32, p % 32
            ddiag[p, c, i, j] = dsum_local[c * P + p]
    return ddiag.reshape(P, cfg.CHT * 4 * 32)


def build_program(cfg: Cfg) -> bass.Bass:
    cfg.check()
    P = 128
    TOK, L, M = cfg.TOK, cfg.L, cfg.M
    DC, CHT, P_CH = cfg.DC, cfg.CHT, cfg.P_CH
    FCH = cfg.FCH
    MT = M // P               # m tiles
    E, R, N = cfg.E, cfg.R, cfg.N
    LCH = L // FCH            # matmul chunks per batch
    TBB = L // P              # token blocks per batch

    nc = bacc.Bacc(
        "TRN2", target_bir_lowering=False, debug=False, num_devices=cfg.n_cores
    )

    # ---- kernel I/O ----
    x_d = nc.dram_tensor("x", [TOK, M], DT, kind="ExternalInput")
    winuT_d = nc.dram_tensor("winuT", [M, DC], BF, kind="ExternalInput")
    winrT_d = nc.dram_tensor("winrT", [M, DC], BF, kind="ExternalInput")
    wconv_d = nc.dram_tensor("wconv", [P, CHT * cfg.KC], DT, kind="ExternalInput")
    bconv_d = nc.dram_tensor("bconv", [P, CHT], DT, kind="ExternalInput")
    wxT_d = {d: nc.dram_tensor(f"wx{d}T", [DC, E], F32R, kind="ExternalInput")
             for d in "fb"}
    wdtT_d = {d: nc.dram_tensor(f"wdt{d}T", [R, DC], F32R, kind="ExternalInput")
              for d in "fb"}
    bdt_d = {d: nc.dram_tensor(f"bdt{d}", [P, CHT], DT, kind="ExternalInput")
             for d in "fb"}
    acol_d = {d: nc.dram_tensor(f"acol{d}", [P, CHT * cfg.TPC], DT,
                                kind="ExternalInput")
              for d in "fb"}
    woutT_d = nc.dram_tensor("woutT", [DC, M], BF, kind="ExternalInput")
    ident_d = nc.dram_tensor("ident", [P, P], DT, kind="ExternalInput")
    rall32_d = nc.dram_tensor("rall32", [P, 4 * P], BF, kind="ExternalInput")
    tsel_d = nc.dram_tensor("tsel", [2 * N, 2 * P], F32R, kind="ExternalInput")
    sall32_d = nc.dram_tensor("sall32", [P, 4 * 32], BF, kind="ExternalInput")
    ddiag_d = nc.dram_tensor("ddiag", [P, CHT * 4 * 32], BF,
                             kind="ExternalInput")

    out_d = nc.dram_tensor("out_rs", [TOK // cfg.n_cores, M], BF,
                           kind="ExternalOutput")

    rg = [list(range(cfg.n_cores))]
    cc_space = "Shared" if cfg.n_cores > 4 else "Local"
    # ReduceScatter chunk schedule (token offset, size): the final chunks are
    # smaller so the exposed tail after the last y is short.
    RS_CHUNKS = [(0, 512), (512, 512), (1024, 1024)]
    RS_OF_B = {0: [0, 1], 1: [2]}

    with tile.TileContext(nc) as tc:
        with tc.tile_pool(name="persist", bufs=1) as pp, \
             tc.tile_pool(name="dram", bufs=1, space="DRAM") as dp:

            # ---------- persistent SBUF (small weights) --------------------
            ident_s = pp.tile([P, P], DT)
            nc.sync.dma_start(ident_s[:], ident_d.ap())
            identb_s = pp.tile([P, P], BF)
            nc.scalar.copy(identb_s[:], ident_s[:])
            rall32_s = pp.tile([P, 4, P], BF)
            nc.sync.dma_start(rall32_s[:], rall32_d.ap().rearrange(
                "p (a b) -> p a b", a=4))
            tsel_s = pp.tile([2 * N, 2, P], F32R)
            nc.sync.dma_start(tsel_s[:], tsel_d.ap().rearrange(
                "k (a b) -> k a b", a=2))
            sall32_s = pp.tile([P, 4, 32], BF)
            nc.sync.dma_start(sall32_s[:], sall32_d.ap().rearrange(
                "p (a b) -> p a b", a=4))
            ddiag_s = pp.tile([P, CHT, 4, 32], BF)
            nc.sync.dma_start(ddiag_s[:], ddiag_d.ap().rearrange(
                "p (c a b) -> p c a b", c=CHT, a=4))
            wconv_s = pp.tile([P, CHT, cfg.KC], DT)
            nc.sync.dma_start(wconv_s[:], wconv_d.ap().rearrange(
                "p (c k) -> p c k", c=CHT))
            bconv_s = pp.tile([P, CHT], DT)
            nc.sync.dma_start(bconv_s[:], bconv_d.ap())
            wx_s, wdt_s, bdt_s, acol_s = {}, {}, {}, {}
            for d in "fb":
                wx_s[d] = pp.tile([P_CH, CHT, E], F32R, name=f"wx{d}_s")
                nc.sync.dma_start(wx_s[d][:], wxT_d[d].ap().rearrange(
                    "(c p) e -> p c e", p=P_CH))
                wdt_s[d] = pp.tile([R, DC], F32R, name=f"wdt{d}_s")
                nc.sync.dma_start(wdt_s[d][:], wdtT_d[d].ap())
                bdt_s[d] = pp.tile([P, CHT], DT, name=f"bdt{d}_s")
                nc.sync.dma_start(bdt_s[d][:], bdt_d[d].ap())
                acol_s[d] = pp.tile([P, CHT * cfg.TPC], DT, name=f"acol{d}_s")
                nc.sync.dma_start(acol_s[d][:], acol_d[d].ap())
            wout_s = pp.tile([P_CH, CHT, M], BF)
            nc.sync.dma_start(wout_s[:], woutT_d.ap().rearrange(
                "(c p) m -> p c m", p=P_CH))

            u_c = [pp.tile([P_CH, TOK], F32R, name=f"u_c{c}") for c in range(CHT)]
            u_bf = [pp.tile([P_CH, TOK], BF, name=f"u_bf{c}") for c in range(CHT)]
            sres = [pp.tile([P_CH, TOK], BF, name=f"sres{c}")
                    for c in range(CHT)]
            y_f = [pp.tile([P_CH, TOK], BF, name=f"y_f{c}") for c in range(CHT)]

            # per-batch dbc partials/reductions (both directions merged)
            dbc_part = [dp.tile([2 * E, L], DT, name=f"dbc_part{b}")
                        for b in range(cfg.B)]
            dbc_red = [dp.tile([2 * E, L], DT, addr_space=cc_space,
                               name=f"dbc_red{b}") for b in range(cfg.B)]
            dbc_red0 = {d: dp.tile([E, L], DT, addr_space=cc_space,
                                   name=f"dbc_red0{d}") for d in "fb"}
            out_part = [dp.tile([sz, M], BF, name=f"out_part{r}")
                        for r, (_, sz) in enumerate(RS_CHUNKS)]
            out_rs = [dp.tile([sz // cfg.n_cores, M], BF, name=f"out_rs{r}")
                      for r, (_, sz) in enumerate(RS_CHUNKS)]

            # ================= front-end (per batch) + res-proj ============
            with tc.tile_pool(name="proj", bufs=1) as jp, \
                 tc.tile_pool(name="proj_ps", bufs=1, space="PSUM") as jpp:
                xT = [jp.tile([P, TOK], BF, name=f"xT{mt}") for mt in range(MT)]
                win_s = jp.tile([P, MT, 2 * DC], BF)
                nc.sync.dma_start(win_s[:, :, :DC], winuT_d.ap().rearrange(
                    "(a p) c -> p a c", p=P))
                nc.sync.dma_start(win_s[:, :, DC:], winrT_d.ap().rearrange(
                    "(a p) c -> p a c", p=P))

                upad = [[jp.tile([P_CH, cfg.KC - 1 + L], DT,
                                 name=f"upad{c}_{b}")
                         for b in range(cfg.B)] for c in range(CHT)]

                TPG = min(4, MT)
                for b in range(cfg.B):
                    bo = b * L
                    for half in range(LCH):
                        t0 = b * TBB + half * (FCH // P)
                        for tb in range(t0, t0 + FCH // P):
                            xsb = jp.tile([P, M], DT, tag="xsb", bufs=2,
                                          name="xsb")
                            nc.sync.dma_start(
                                xsb[:], x_d.ap()[tb * P:(tb + 1) * P, :])
                            xbb = jp.tile([P, M], BF, tag="xbb", bufs=2,
                                          name="xbb")
                            nc.scalar.copy(xbb[:], xsb[:])
                            for mg in range(MT // TPG):
                                tp_ps = jpp.tile([P, TPG * P], BF, tag="tp",
                                                 bufs=4, name="tp_ps")
                                for k in range(TPG):
                                    mt = mg * TPG + k
                                    nc.tensor.transpose(
                                        tp_ps[:, k * P:(k + 1) * P],
                                        xbb[:, mt * P:(mt + 1) * P],
                                        identb_s[:])
                                for k in range(TPG):
                                    mt = mg * TPG + k
                                    nc.vector.tensor_copy(
                                        xT[mt][:, tb * P:(tb + 1) * P],
                                        tp_ps[:, k * P:(k + 1) * P])
                        f0 = half * FCH
                        for c in range(CHT):
                            ups = jpp.tile([P_CH, FCH], DT, tag="mm", bufs=4,
                                           name="ups")
                            for kt in range(MT):
                                nc.tensor.matmul(
                                    ups[:],
                                    win_s[:, kt, c * P_CH:(c + 1) * P_CH],
                                    xT[kt][:, bo + f0:bo + f0 + FCH],
                                    start=(kt == 0), stop=(kt == MT - 1))
                            nc.scalar.copy(
                                upad[c][b][:, cfg.KC - 1 + f0:
                                           cfg.KC - 1 + f0 + FCH], ups[:])

                    for c in range(CHT):
                        nc.gpsimd.memset(upad[c][b][:, :cfg.KC - 1], 0.0)
                        # depthwise causal conv + SiLU
                        acc = None
                        for k in range(cfg.KC):
                            nxt = jp.tile([P_CH, L], DT, tag="cacc",
                                          bufs=2, name="cacc")
                            tap = upad[c][b][:, k:k + L]
                            wk = wconv_s[:P_CH, c, k:k + 1]
                            if acc is None:
                                nc.vector.tensor_scalar(
                                    nxt[:], tap, wk,
                                    bconv_s[:P_CH, c:c + 1],
                                    OP.mult, OP.add)
                            else:
                                nc.vector.scalar_tensor_tensor(
                                    nxt[:], tap, wk, acc[:],
                                    OP.mult, OP.add)
                            acc = nxt
                        sg2 = jp.tile([P_CH, L], DT, tag="sg2", bufs=2,
                                      name="sg2")
                        nc.scalar.activation(sg2[:], acc[:], AF.Sigmoid)
                        nc.gpsimd.tensor_tensor(
                            u_c[c][:, bo:bo + L], acc[:], sg2[:], OP.mult)
                        nc.scalar.copy(
                            u_bf[c][:, bo:bo + L],
                            u_c[c][:, bo:bo + L].bitcast(DT))

                    # dbc partials (both dirs) for this batch -> AllReduce(b)
                    for di, d in enumerate("fb"):
                        for lh in range(LCH):
                            f0 = lh * FCH
                            bps = jpp.tile([E, FCH], DT, tag="mm", bufs=4,
                                           name="bps")
                            for c in range(CHT):
                                nc.tensor.matmul(
                                    bps[:],
                                    wx_s[d][:, c, :],
                                    u_c[c][:, bo + f0:bo + f0 + FCH],
                                    start=(c == 0), stop=(c == CHT - 1))
                            bst = jp.tile([E, FCH], DT, tag="bst", bufs=3,
                                          name="bst")
                            nc.scalar.copy(bst[:], bps[:])
                            nc.sync.dma_start(
                                dbc_part[b][di * E:(di + 1) * E,
                                            f0:f0 + FCH], bst[:])
                    if b == 0:
                        # split by direction: the f half lands earlier so the
                        # scan phase starts sooner; the b half hides under it
                        nc.gpsimd.collective_compute(
                            "AllReduce", OP.add, replica_groups=rg,
                            ins=[dbc_part[b][:E, :].opt()],
                            outs=[dbc_red0["f"].opt()])
                        nc.gpsimd.collective_compute(
                            "AllReduce", OP.add, replica_groups=rg,
                            ins=[dbc_part[b][E:, :].opt()],
                            outs=[dbc_red0["b"].opt()])
                    else:
                        nc.gpsimd.collective_compute(
                            "AllReduce", OP.add, replica_groups=rg,
                            ins=[dbc_part[b].opt()], outs=[dbc_red[b].opt()])

                # res projection (overlaps the AllReduces)
                for c in range(CHT):
                    for fc in range(TOK // FCH):
                        f0 = fc * FCH
                        rps = jpp.tile([P_CH, FCH], DT, tag="mm", bufs=4,
                                       name="rps")
                        for kt in range(MT):
                            nc.tensor.matmul(
                                rps[:],
                                win_s[:, kt, DC + c * P_CH:DC + (c + 1) * P_CH],
                                xT[kt][:, f0:f0 + FCH],
                                start=(kt == 0), stop=(kt == MT - 1))
                        sg = jp.tile([P_CH, FCH], BF, tag="sg", bufs=2,
                                     name="sg")
                        nc.scalar.activation(sg[:], rps[:], AF.Sigmoid)
                        nc.vector.tensor_tensor(sres[c][:, f0:f0 + FCH],
                                                rps[:], sg[:], OP.mult)

            # ================= scan phase + out_proj (per batch) ===========
            with tc.tile_pool(name="scan_sb", bufs=1) as sp, \
                 tc.tile_pool(name="scan_ps", bufs=1, space="PSUM") as spp, \
                 tc.tile_pool(name="comb", bufs=1) as kp:

                def rep_tile():
                    return spp.tile([P, 2 * FCH], DT, tag="rep", bufs=3,
                                    name="rep")

                for b in range(cfg.B):
                    bo = b * L
                    for di, d in enumerate("fb"):
                        off = di * E
                        src = dbc_red0[d] if b == 0 else dbc_red[b]
                        soff = 0 if b == 0 else off
                        dt_sb = sp.tile([R, L], F32R, tag="dt", bufs=2,
                                        name=f"dt_{d}{b}")
                        nc.sync.dma_start(
                            dt_sb[:], src[soff:soff + R, :].bitcast(F32R))
                        bc_sb = sp.tile([2 * N, L], F32R, tag="bc", bufs=2,
                                        name=f"bc_{d}{b}")
                        nc.sync.dma_start(
                            bc_sb[:],
                            src[soff + R:soff + E, :].bitcast(F32R))

                        # B/C replicated across the 8-channel groups
                        brep = sp.tile([P, L], BF, tag="brep", bufs=2,
                                       name=f"brep{d}")
                        crep = sp.tile([P, L], BF, tag="crep", bufs=2,
                                       name=f"crep{d}")
                        for which, rep in ((0, brep), (1, crep)):
                            ps = rep_tile()
                            for lh in range(LCH):
                                o = lh * FCH
                                nc.tensor.matmul(
                                    ps[:, o:o + FCH],
                                    tsel_s[:, which, :],
                                    bc_sb[:, o:o + FCH],
                                    start=True, stop=True)
                            nc.scalar.copy(rep[:], ps[:])

                        # delta = softplus(dt @ WdtT + bdt); w = delta * u
                        delta = [sp.tile([P_CH, L], BF, tag=f"delta{c}",
                                         bufs=2, name=f"delta_{d}{c}")
                                 for c in range(CHT)]
                        w_s = [sp.tile([P_CH, L], BF, tag=f"w{c}", bufs=2,
                                       name=f"w_{d}{c}") for c in range(CHT)]
                        for c in range(CHT):
                            ps = rep_tile()
                            for lh in range(LCH):
                                o = lh * FCH
                                nc.tensor.matmul(
                                    ps[:, o:o + FCH],
                                    wdt_s[d][:, c * P_CH:(c + 1) * P_CH],
                                    dt_sb[:, o:o + FCH],
                                    start=True, stop=True)
                            spt = sp.tile([P_CH, L], DT, tag="spt", bufs=2,
                                          name="spt")
                            # softplus(x + bdt) = ln(1 + exp(x + bdt))
                            nc.scalar.activation(
                                spt[:], ps[:], AF.Exp,
                                bias=bdt_s[d][:P_CH, c:c + 1])
                            nc.scalar.activation(
                                delta[c][:], spt[:], AF.Ln, bias=1.0)
                            nc.vector.tensor_tensor(
                                w_s[c][:], delta[c][:],
                                u_bf[c][:, bo:bo + L], OP.mult)

                        for c in range(CHT):
                            hC = [None] * 16
                            for k in range(4):      # concurrent quad sets
                                dA, dBu = {}, {}
                                dpq, wpq = {}, {}
                                for i in range(4):
                                    g = 4 * i + k
                                    dpq[g] = rep_tile()
                                    for lh in range(LCH):
                                        q = lh * FCH
                                        nc.tensor.matmul(
                                            dpq[g][:, q:q + FCH],
                                            rall32_s[32 * i:32 * i + 32,
                                                     k, :],
                                            delta[c][32 * i:32 * i + 32,
                                                     q:q + FCH],
                                            start=True, stop=True,
                                            tile_position=(32 * i, 0))
                                for i in range(4):
                                    g = 4 * i + k
                                    j = c * 16 + g
                                    dA[g] = sp.tile([P, L], BF, tag="dA",
                                                    bufs=6, name="dA")
                                    nc.scalar.activation(
                                        dA[g][:], dpq[g][:], AF.Exp,
                                        scale=acol_s[d][:, j:j + 1])
                                for i in range(4):
                                    g = 4 * i + k
                                    wpq[g] = rep_tile()
                                    for lh in range(LCH):
                                        q = lh * FCH
                                        nc.tensor.matmul(
                                            wpq[g][:, q:q + FCH],
                                            rall32_s[32 * i:32 * i + 32,
                                                     k, :],
                                            w_s[c][32 * i:32 * i + 32,
                                                   q:q + FCH],
                                            start=True, stop=True,
                                            tile_position=(32 * i, 0))
                                for i in range(4):
                                    g = 4 * i + k
                                    dBu[g] = sp.tile([P, L], BF, tag="dBu",
                                                     bufs=6, name="dBu")
                                    if i == 0:
                                        nc.vector.tensor_tensor(
                                            dBu[g][:], wpq[g][:], brep[:],
                                            OP.mult)
                                    else:
                                        wsb = sp.tile([P, L], BF, tag="wsb",
                                                      bufs=3, name="wsb")
                                        nc.scalar.copy(wsb[:], wpq[g][:])
                                        nc.vector.tensor_tensor(
                                            dBu[g][:], wsb[:], brep[:],
                                            OP.mult)
                                for i in range(4):
                                    g = 4 * i + k
                                    h = sp.tile([P, L], BF, tag="h", bufs=10,
                                                name="h")
                                    if d == "f":
                                        nc.vector.tensor_tensor_scan(
                                            h[:], dA[g][:], dBu[g][:], 0.0,
                                            OP.mult, OP.add)
                                    else:
                                        nc.vector.tensor_tensor_scan(
                                            h[:, ::-1], dA[g][:, ::-1],
                                            dBu[g][:, ::-1],
                                            0.0, OP.mult, OP.add)
                                    hC[g] = sp.tile([P, L], BF, tag="hC",
                                                    bufs=17, name="hC")
                                    nc.vector.tensor_tensor(
                                        hC[g][:], h[:], crep[:], OP.mult)
                            # column-tiled y reduction over all 16 groups
                            y_ps = spp.tile([P_CH, L], DT, tag="y", bufs=1,
                                            name="y_ps")
                            for lh in range(LCH):
                                q = lh * FCH
                                for k in range(4):
                                    for i in range(4):
                                        g = 4 * i + k
                                        nc.tensor.matmul(
                                            y_ps[32 * i:32 * i + 32,
                                                 q:q + FCH],
                                            sall32_s[:, k, :],
                                            hC[g][:, q:q + FCH],
                                            start=(k == 0),
                                            stop=(k == 3 and d == "f"),
                                            tile_position=(0, 32 * i))
                                if d == "b":
                                    # add u*(fD+bD) into the accumulation
                                    for i in range(4):
                                        nc.tensor.matmul(
                                            y_ps[32 * i:32 * i + 32,
                                                 q:q + FCH],
                                            ddiag_s[:, c, i, :],
                                            u_bf[c][:, bo + q:bo + q + FCH],
                                            start=False, stop=True,
                                            tile_position=(0, 32 * i))
                            if d == "f":
                                nc.scalar.copy(y_f[c][:, bo:bo + L], y_ps[:])
                            else:
                                # fused combine:
                                # y = (y_f + y_b + u*(fD+bD)) * (0.5*silu(res))
                                # (the 0.5 is folded into W_out host-side)
                                ysl = y_f[c][:, bo:bo + L]
                                t1 = kp.tile([P_CH, L], BF, tag="t5", bufs=2,
                                             name="t1")
                                nc.vector.tensor_tensor(t1[:], y_ps[:],
                                                        ysl, OP.add)
                                nc.vector.tensor_tensor(
                                    ysl, t1[:], sres[c][:, bo:bo + L],
                                    OP.mult)

                    # out_proj + ReduceScatter for this batch
                    MFC = min(512, M)
                    for r in RS_OF_B[b]:
                        toff, sz = RS_CHUNKS[r]
                        for tr in range(sz // P):
                            t0 = toff + tr * P
                            ops = spp.tile([P, M], DT,
                                           tag=("rep" if b == 1 else "y"),
                                           bufs=(3 if b == 1 else 1),
                                           name="ops")
                            for mc in range(M // MFC):
                                o = mc * MFC
                                for c in range(CHT):
                                    nc.tensor.matmul(
                                        ops[:, o:o + MFC],
                                        y_f[c][:, t0:t0 + P],
                                        wout_s[:, c, o:o + MFC],
                                        start=(c == 0), stop=(c == CHT - 1))
                            ost = kp.tile([P, M], BF, tag="ost", bufs=3,
                                          name="ost")
                            nc.scalar.copy(ost[:], ops[:])
                            nc.sync.dma_start(
                                out_part[r][tr * P:(tr + 1) * P, :], ost[:])
                        nc.gpsimd.collective_compute(
                            "ReduceScatter", OP.add, replica_groups=rg,
                            ins=[out_part[r].opt()], outs=[out_rs[r].opt()])
                        nc.sync.dma_start(
                            out_d.ap()[toff // cfg.n_cores:
                                       (toff + sz) // cfg.n_cores, :],
                            out_rs[r][:])

    nc.compile()
    return nc


# --------------------------------------------------------------------------
# host side
# --------------------------------------------------------------------------

def host_prep(cfg: Cfg, inputs: dict) -> list[dict]:
    """Slice the full-model inputs into one input map per core."""
    P = 128
    f32 = np.float32

    def g(name):
        return np.asarray(inputs[name], f32)

    x = g("x").reshape(cfg.TOK, cfg.M)
    W_in = g("W_in")
    W_conv = g("W_conv").reshape(cfg.DI, cfg.KC)
    b_conv = g("b_conv")
    W_out = g("W_out")
    ident, rall32, t_sel, sall32 = build_consts(cfg)
    tsel_flat = t_sel.reshape(2 * cfg.N, 2 * P)
    sall32_flat = sall32.reshape(P, 4 * 32)
    rall32_flat = rall32.reshape(P, 4 * P)

    per = {}
    for d in "fb":
        per[d] = dict(
            A=-np.exp(g(d + "A_log")),            # (DI, N)
            D=g(d + "D"),
            Wx=g(d + "Wx"),                       # (E, DI)
            Wdt=g(d + "Wdt"),                     # (DI, R)
            bdt=g(d + "bdt"),
        )

    def col_layout(v):  # (DC,) -> (P_CH, CHT): [p, c] = v[c*P_CH + p]
        return np.ascontiguousarray(
            v.reshape(cfg.CHT, cfg.P_CH).T.astype(f32))

    def pad_p(a):  # pad partition dim up to 128
        if a.shape[0] == P:
            return np.ascontiguousarray(a.astype(f32))
        out = np.zeros((P,) + a.shape[1:], f32)
        out[:a.shape[0]] = a
        return out

    in_maps = []
    for core in range(cfg.n_cores):
        c0 = core * cfg.DC
        ch = slice(c0, c0 + cfg.DC)
        m = {
            "x": x,
            "winuT": np.ascontiguousarray(
                W_in[ch, :].T.astype(ml_dtypes.bfloat16)),
            "winrT": np.ascontiguousarray(
                W_in[cfg.DI + c0:cfg.DI + c0 + cfg.DC, :]
                .T.astype(ml_dtypes.bfloat16)),
            "wconv": pad_p(
                W_conv[ch].reshape(cfg.CHT, cfg.P_CH, cfg.KC)
                .transpose(1, 0, 2).reshape(cfg.P_CH, cfg.CHT * cfg.KC)),
            "bconv": pad_p(col_layout(b_conv[ch])),
            "woutT": np.ascontiguousarray(
                (W_out[:, ch].T * 0.5).astype(ml_dtypes.bfloat16)),
            "ident": ident,
            "ddiag": build_ddiag(
                cfg, (per["f"]["D"][ch] + per["b"]["D"][ch]).astype(f32)
            ).astype(ml_dtypes.bfloat16),
            "rall32": rall32_flat.astype(ml_dtypes.bfloat16),
            "tsel": tsel_flat,
            "sall32": sall32_flat.astype(ml_dtypes.bfloat16),
        }
        for d in "fb":
            pd = per[d]
            m[f"wx{d}T"] = np.ascontiguousarray(pd["Wx"][:, ch].T)
            m[f"wdt{d}T"] = np.ascontiguousarray(pd["Wdt"][ch, :].T)
            m[f"bdt{d}"] = pad_p(col_layout(pd["bdt"][ch]))
            # A columns: [p, j] = A[8j + p//16, p%16] (local channels)
            Ac = pd["A"][ch]                       # (DC, N)
            acol = np.empty((P, cfg.NT), f32)
            pidx = np.arange(P)
            for j in range(cfg.NT):
                acol[:, j] = Ac[8 * j + pidx // 16, pidx % 16]
            m[f"acol{d}"] = acol
        in_maps.append({k: np.ascontiguousarray(v) for k, v in m.items()})
    return in_maps


RS_CHUNKS_HOST = [(0, 512), (512, 512), (1024, 1024)]


def gather_out(cfg: Cfg, results: list[dict]) -> np.ndarray:
    out = np.empty((cfg.TOK, cfg.M), np.float32)
    for core in range(cfg.n_cores):
        shard = np.asarray(results[core]["out_rs"])  # (TOK//n_cores, M)
        cum = 0
        for toff, sz in RS_CHUNKS_HOST:
            sh = sz // cfg.n_cores
            out[toff + core * sh:toff + (core + 1) * sh, :] = \
                shard[cum:cum + sh, :]
            cum += sh
    return out.reshape(cfg.B, cfg.L, cfg.M).astype(np.float32)


def kernel(**inputs) -> np.ndarray:
    cfg = FULL
    from concourse.bass_utils import run_bass_kernel_spmd
    nc = build_program(cfg)
    in_maps = host_prep(cfg, inputs)
    res = run_bass_kernel_spmd(nc, in_maps, core_ids=list(range(cfg.n_cores)))
    return gather_out(cfg, res.results)


# revision 89
# speedup vs baseline: 1.1529x; 1.0033x over previous
"""Bidirectional Mamba block (in_proj -> depthwise causal conv -> SiLU ->
forward+backward S6 selective scan -> gated combine -> out_proj) as a
Trainium2 Bass/Tile SPMD kernel over 8 NeuronCores.

Sharding: tensor-parallel over d_inner (256 channels per core). The conv and
the S6 scans are channel-independent, so they need no communication.

Collectives are pipelined per batch element:
  * AllReduce(b) of both directions' partial x-projections dbc = u @ Wx^T
    ([192, 1024] fp32) is issued as soon as batch b's conv is done; AR(b0)
    overlaps the b1 front-end and the res projection, AR(b1) overlaps the
    b0 scan phase.
  * AR(b0) is further split by direction so the forward half (which gates
    the first scan) lands earlier.
  * The partial out-projection is cast to bf16 and ReduceScattered: two
    chunks for b0 (they overlap the b1 scan phase) but a single chunk for
    b1, whose only cost is launch+ring latency at the tail.  (More or
    uneven chunks measured slower: each extra exposed collective costs
    tens of microseconds of serialized launch/ring time.)

Scan layout: partitions = (8 channels x 16 states), free dim = L, one scan
instruction per (direction, batch, channel-group) on the DVE hardware scan
(fp32 internal state; the per-element feedback bubble pins it at ~2.2
cycles/element regardless of dtype).  The x transposes, in_proj and the
whole scan phase run in bf16 (rel err ~6e-3 vs the 2e-2 budget).

The entire dBu -> scan -> hC chain runs on the vector engine: for 3 of the
4 groups in a quad the scalar engine first evacuates the PE-produced w
replica to bf16 SBUF so the dBu multiply runs at the DVE 2x rate; the
fourth multiplies straight out of PSUM (1x) to keep the scalar engine
under its saturation point.  hC runs at the 2x bf16 rate.  Offloading
dBu/hC to GpSimd measures *slower* -- a GpSimd tensor op co-running with a
scan inflates the scan ~1.8us via SBUF port contention, more than the
offload saves.

PE array tiling: the delta/w replication matmuls have a true contraction of
8 rows, so four channel groups run concurrently in 32x128 row-tiles
(tile_position=(32i, 0), 32-row stationaries selecting 8 rows); the
per-group state reduction y = sum_n C*h has 8 output rows, so four groups
run concurrently in 128x32 column tiles (tile_position=(0, 32i)), and the
skip term u*(fD+bD) is folded into the same PSUM accumulation as a fifth,
diagonal column-tiled matmul.  Channel group g = 4i+k lives at partition
base 32i+8k, which places every concurrent quad {k, 4+k, 8+k, 12+k} in four
distinct array quadrants without any host-side channel permutation.
"""

import os
import sys

for _p in ("/opt/trn_rl_repo", "/root/.axon_site/_ro/trn_rl_repo"):
    if os.path.isdir(_p) and _p not in sys.path:
        sys.path.append(_p)

from dataclasses import dataclass

import ml_dtypes
import numpy as np

import concourse.bass as bass
import concourse.mybir as mybir
import concourse.tile as tile
from concourse import bacc

DT = mybir.dt.float32
F32R = mybir.dt.float32r
BF = mybir.dt.bfloat16
AF = mybir.ActivationFunctionType
OP = mybir.AluOpType


@dataclass(frozen=True)
class Cfg:
    n_cores: int = 8
    B: int = 2
    L: int = 1024
    M: int = 1024      # d_model
    DI: int = 2048     # d_inner
    N: int = 16        # d_state
    R: int = 64        # dt_rank
    KC: int = 4        # conv kernel

    @property
    def DC(self):  # channels per core
        return self.DI // self.n_cores

    @property
    def TOK(self):
        return self.B * self.L

    @property
    def P_CH(self):  # partitions per channel tile
        return min(128, self.DC)

    @property
    def CHT(self):  # channel tiles per core
        return self.DC // self.P_CH

    @property
    def NT(self):  # scan tiles per (dir, batch): 8 channels each
        return self.DC // 8

    @property
    def TPC(self):  # scan tiles per channel tile
        return self.P_CH // 8

    @property
    def FCH(self):  # matmul moving-dim chunk over tokens
        return min(512, self.L)

    @property
    def E(self):
        return self.R + 2 * self.N

    def check(self):
        assert self.DC % 8 == 0 and self.DC % self.P_CH == 0
        assert self.M % 128 == 0
        assert self.TOK % 128 == 0 and self.TOK % self.FCH == 0
        assert self.L % min(512, self.L) == 0
        assert self.N == 16 and self.TPC == 16


FULL = Cfg()


def build_consts(cfg: Cfg):
    """Selection matrices used as PE 'weights' (exact 0/1 values)."""
    P = 128
    ident = np.eye(P, dtype=np.float32)
    # rall32[32i+r, k, col] = 1 iff r in [8k, 8k+8) and col//16 == r-8k:
    # a 32-row stationary (32-aligned, as the PE tiling requires) that
    # replicates the 8 channel rows of group 4i+k into 128 (ch x state)
    # rows; the other 24 rows of the quadrant contribute zeros.
    rall32 = np.zeros((P, 4, P), np.float32)
    for p in range(P):
        r = p % 32
        for k in range(4):
            if 8 * k <= r < 8 * k + 8:
                rall32[p, k, (r - 8 * k) * 16:(r - 8 * k + 1) * 16] = 1.0
    # tsel[:, which, :]: out[p] = src[16*which + p%16]  (B/C replication)
    t_sel = np.zeros((2 * cfg.N, 2, P), np.float32)
    for which in range(2):
        for p in range(P):
            t_sel[cfg.N * which + p % 16, which, p] = 1.0
    # sall32[p, k, j] = 1 iff j == 8k + p//16: reduce the 16 state rows of
    # channel e into output row 8k+e of a 32-row column tile.
    sall32 = np.zeros((P, 4, 32), np.float32)
    for p in range(P):
        for k in range(4):
            sall32[p, k, 8 * k + p // 16] = 1.0
    # ddiag[p, c, i, j] = dsum[c*128+p] iff p == 32i+j: adds u*(fD+bD) into
    # the y accumulation as a 4-way column-tiled diagonal matmul.
    return ident, rall32, t_sel, sall32


def build_ddiag(cfg: Cfg, dsum_local: np.ndarray):
    P = 128
    ddiag = np.zeros((P, cfg.CHT, 4, 32), np.float32)
    for c in range(cfg.CHT):
        for p in range(P):
            i, j = p // 32, p % 32
            ddiag[p, c, i, j] = dsum_local[c * P + p]
    return ddiag.reshape(P, cfg.CHT * 4 * 32)


def build_program(cfg: Cfg) -> bass.Bass:
    cfg.check()
    P = 128
    TOK, L, M = cfg.TOK, cfg.L, cfg.M
    DC, CHT, P_CH = cfg.DC, cfg.CHT, cfg.P_CH
    FCH = cfg.FCH
    MT = M // P               # m tiles
    E, R, N = cfg.E, cfg.R, cfg.N
    LCH = L // FCH            # matmul chunks per batch
    TBB = L // P              # token blocks per batch

    nc = bacc.Bacc(
        "TRN2", target_bir_lowering=False, debug=False, num_devices=cfg.n_cores
    )

    # ---- kernel I/O ----
    x_d = nc.dram_tensor("x", [TOK, M], DT, kind="ExternalInput")
    winuT_d = nc.dram_tensor("winuT", [M, DC], BF, kind="ExternalInput")
    winrT_d = nc.dram_tensor("winrT", [M, DC], BF, kind="ExternalInput")
    wconv_d = nc.dram_tensor("wconv", [P, CHT * cfg.KC], DT, kind="ExternalInput")
    bconv_d = nc.dram_tensor("bconv", [P, CHT], DT, kind="ExternalInput")
    wxT_d = {d: nc.dram_tensor(f"wx{d}T", [DC, E], F32R, kind="ExternalInput")
             for d in "fb"}
    wdtT_d = {d: nc.dram_tensor(f"wdt{d}T", [R, DC], F32R, kind="ExternalInput")
              for d in "fb"}
    bdt_d = {d: nc.dram_tensor(f"bdt{d}", [P, CHT], DT, kind="ExternalInput")
             for d in "fb"}
    acol_d = {d: nc.dram_tensor(f"acol{d}", [P, CHT * cfg.TPC], DT,
                                kind="ExternalInput")
              for d in "fb"}
    woutT_d = nc.dram_tensor("woutT", [DC, M], BF, kind="ExternalInput")
    ident_d = nc.dram_tensor("ident", [P, P], DT, kind="ExternalInput")
    rall32_d = nc.dram_tensor("rall32", [P, 4 * P], BF, kind="ExternalInput")
    tsel_d = nc.dram_tensor("tsel", [2 * N, 2 * P], F32R, kind="ExternalInput")
    sall32_d = nc.dram_tensor("sall32", [P, 4 * 32], BF, kind="ExternalInput")
    ddiag_d = nc.dram_tensor("ddiag", [P, CHT * 4 * 32], BF,
                             kind="ExternalInput")

    out_d = nc.dram_tensor("out_rs", [TOK // cfg.n_cores, M], BF,
                           kind="ExternalOutput")

    rg = [list(range(cfg.n_cores))]
    cc_space = "Shared" if cfg.n_cores > 4 else "Local"
    # ReduceScatter chunk schedule (token offset, size): the final chunks are
    # smaller so the exposed tail after the last y is short.
    RS_CHUNKS = [(0, 512), (512, 512), (1024, 1024)]
    RS_OF_B = {0: [0, 1], 1: [2]}

    with tile.TileContext(nc) as tc:
        with tc.tile_pool(name="persist", bufs=1) as pp, \
             tc.tile_pool(name="dram", bufs=1, space="DRAM") as dp:

            # ---------- persistent SBUF (small weights) --------------------
            ident_s = pp.tile([P, P], DT)
            nc.sync.dma_start(ident_s[:], ident_d.ap())
            identb_s = pp.tile([P, P], BF)
            nc.scalar.copy(identb_s[:], ident_s[:])
            rall32_s = pp.tile([P, 4, P], BF)
            nc.sync.dma_start(rall32_s[:], rall32_d.ap().rearrange(
                "p (a b) -> p a b", a=4))
            tsel_s = pp.tile([2 * N, 2, P], F32R)
            nc.sync.dma_start(tsel_s[:], tsel_d.ap().rearrange(
                "k (a b) -> k a b", a=2))
            sall32_s = pp.tile([P, 4, 32], BF)
            nc.sync.dma_start(sall32_s[:], sall32_d.ap().rearrange(
                "p (a b) -> p a b", a=4))
            ddiag_s = pp.tile([P, CHT, 4, 32], BF)
            nc.sync.dma_start(ddiag_s[:], ddiag_d.ap().rearrange(
                "p (c a b) -> p c a b", c=CHT, a=4))
            wconv_s = pp.tile([P, CHT, cfg.KC], DT)
            nc.sync.dma_start(wconv_s[:], wconv_d.ap().rearrange(
                "p (c k) -> p c k", c=CHT))
            bconv_s = pp.tile([P, CHT], DT)
            nc.sync.dma_start(bconv_s[:], bconv_d.ap())
            wx_s, wdt_s, bdt_s, acol_s = {}, {}, {}, {}
            for d in "fb":
                wx_s[d] = pp.tile([P_CH, CHT, E], F32R, name=f"wx{d}_s")
                nc.sync.dma_start(wx_s[d][:], wxT_d[d].ap().rearrange(
                    "(c p) e -> p c e", p=P_CH))
                wdt_s[d] = pp.tile([R, DC], F32R, name=f"wdt{d}_s")
                nc.sync.dma_start(wdt_s[d][:], wdtT_d[d].ap())
                bdt_s[d] = pp.tile([P, CHT], DT, name=f"bdt{d}_s")
                nc.sync.dma_start(bdt_s[d][:], bdt_d[d].ap())
                acol_s[d] = pp.tile([P, CHT * cfg.TPC], DT, name=f"acol{d}_s")
                nc.sync.dma_start(acol_s[d][:], acol_d[d].ap())
            wout_s = pp.tile([P_CH, CHT, M], BF)
            nc.sync.dma_start(wout_s[:], woutT_d.ap().rearrange(
                "(c p) m -> p c m", p=P_CH))

            u_c = [pp.tile([P_CH, TOK], F32R, name=f"u_c{c}") for c in range(CHT)]
            u_bf = [pp.tile([P_CH, TOK], BF, name=f"u_bf{c}") for c in range(CHT)]
            sres = [pp.tile([P_CH, TOK], BF, name=f"sres{c}")
                    for c in range(CHT)]
            y_f = [pp.tile([P_CH, TOK], BF, name=f"y_f{c}") for c in range(CHT)]

            # per-batch dbc partials/reductions (both directions merged)
            dbc_part = [dp.tile([2 * E, L], DT, name=f"dbc_part{b}")
                        for b in range(cfg.B)]
            dbc_red = [dp.tile([2 * E, L], DT, addr_space=cc_space,
                               name=f"dbc_red{b}") for b in range(cfg.B)]
            dbc_red0 = {d: dp.tile([E, L], DT, addr_space=cc_space,
                                   name=f"dbc_red0{d}") for d in "fb"}
            out_part = [dp.tile([sz, M], BF, name=f"out_part{r}")
                        for r, (_, sz) in enumerate(RS_CHUNKS)]
            out_rs = [dp.tile([sz // cfg.n_cores, M], BF, name=f"out_rs{r}")
                      for r, (_, sz) in enumerate(RS_CHUNKS)]

            # ================= front-end (per batch) + res-proj ============
            with tc.tile_pool(name="proj", bufs=1) as jp, \
                 tc.tile_pool(name="proj_ps", bufs=1, space="PSUM") as jpp:
                xT = [jp.tile([P, TOK], BF, name=f"xT{mt}") for mt in range(MT)]
                win_s = jp.tile([P, MT, 2 * DC], BF)
                nc.sync.dma_start(win_s[:, :, :DC], winuT_d.ap().rearrange(
                    "(a p) c -> p a c", p=P))
                nc.sync.dma_start(win_s[:, :, DC:], winrT_d.ap().rearrange(
                    "(a p) c -> p a c", p=P))

                upad = [[jp.tile([P_CH, cfg.KC - 1 + L], DT,
                                 name=f"upad{c}_{b}")
                         for b in range(cfg.B)] for c in range(CHT)]

                TPG = min(4, MT)
                for b in range(cfg.B):
                    bo = b * L
                    for half in range(LCH):
                        t0 = b * TBB + half * (FCH // P)
                        for tb in range(t0, t0 + FCH // P):
                            xsb = jp.tile([P, M], DT, tag="xsb", bufs=2,
                                          name="xsb")
                            nc.sync.dma_start(
                                xsb[:], x_d.ap()[tb * P:(tb + 1) * P, :])
                            xbb = jp.tile([P, M], BF, tag="xbb", bufs=2,
                                          name="xbb")
                            nc.scalar.copy(xbb[:], xsb[:])
                            for mg in range(MT // TPG):
                                tp_ps = jpp.tile([P, TPG * P], BF, tag="tp",
                                                 bufs=4, name="tp_ps")
                                for k in range(TPG):
                                    mt = mg * TPG + k
                                    nc.tensor.transpose(
                                        tp_ps[:, k * P:(k + 1) * P],
                                        xbb[:, mt * P:(mt + 1) * P],
                                        identb_s[:])
                                for k in range(TPG):
                                    mt = mg * TPG + k
                                    nc.vector.tensor_copy(
                                        xT[mt][:, tb * P:(tb + 1) * P],
                                        tp_ps[:, k * P:(k + 1) * P])
                        f0 = half * FCH
                        for c in range(CHT):
                            ups = jpp.tile([P_CH, FCH], DT, tag="mm", bufs=4,
                                           name="ups")
                            for kt in range(MT):
                                nc.tensor.matmul(
                                    ups[:],
                                    win_s[:, kt, c * P_CH:(c + 1) * P_CH],
                                    xT[kt][:, bo + f0:bo + f0 + FCH],
                                    start=(kt == 0), stop=(kt == MT - 1))
                            nc.scalar.copy(
                                upad[c][b][:, cfg.KC - 1 + f0:
                                           cfg.KC - 1 + f0 + FCH], ups[:])

                    for c in range(CHT):
                        nc.gpsimd.memset(upad[c][b][:, :cfg.KC - 1], 0.0)
                        # depthwise causal conv + SiLU
                        acc = None
                        for k in range(cfg.KC):
                            nxt = jp.tile([P_CH, L], DT, tag="cacc",
                                          bufs=2, name="cacc")
                            tap = upad[c][b][:, k:k + L]
                            wk = wconv_s[:P_CH, c, k:k + 1]
                            if acc is None:
                                nc.vector.tensor_scalar(
                                    nxt[:], tap, wk,
                                    bconv_s[:P_CH, c:c + 1],
                                    OP.mult, OP.add)
                            else:
                                nc.vector.scalar_tensor_tensor(
                                    nxt[:], tap, wk, acc[:],
                                    OP.mult, OP.add)
                            acc = nxt
                        sg2 = jp.tile([P_CH, L], DT, tag="sg2", bufs=2,
                                      name="sg2")
                        nc.scalar.activation(sg2[:], acc[:], AF.Sigmoid)
                        nc.gpsimd.tensor_tensor(
                            u_c[c][:, bo:bo + L], acc[:], sg2[:], OP.mult)
                        nc.scalar.copy(
                            u_bf[c][:, bo:bo + L],
                            u_c[c][:, bo:bo + L].bitcast(DT))

                    # dbc partials (both dirs) for this batch -> AllReduce(b)
                    for di, d in enumerate("fb"):
                        for lh in range(LCH):
                            f0 = lh * FCH
                            bps = jpp.tile([E, FCH], DT, tag="mm", bufs=4,
                                           name="bps")
                            for c in range(CHT):
                                nc.tensor.matmul(
                                    bps[:],
                                    wx_s[d][:, c, :],
                                    u_c[c][:, bo + f0:bo + f0 + FCH],
                                    start=(c == 0), stop=(c == CHT - 1))
                            bst = jp.tile([E, FCH], DT, tag="bst", bufs=3,
                                          name="bst")
                            nc.scalar.copy(bst[:], bps[:])
                            nc.sync.dma_start(
                                dbc_part[b][di * E:(di + 1) * E,
                                            f0:f0 + FCH], bst[:])
                    if b == 0:
                        # split by direction: the f half lands earlier so the
                        # scan phase starts sooner; the b half hides under it
                        nc.gpsimd.collective_compute(
                            "AllReduce", OP.add, replica_groups=rg,
                            ins=[dbc_part[b][:E, :].opt()],
                            outs=[dbc_red0["f"].opt()])
                        nc.gpsimd.collective_compute(
                            "AllReduce", OP.add, replica_groups=rg,
                            ins=[dbc_part[b][E:, :].opt()],
                            outs=[dbc_red0["b"].opt()])
                    else:
                        nc.gpsimd.collective_compute(
                            "AllReduce", OP.add, replica_groups=rg,
                            ins=[dbc_part[b].opt()], outs=[dbc_red[b].opt()])

                # res projection (overlaps the AllReduces)
                for c in range(CHT):
                    for fc in range(TOK // FCH):
                        f0 = fc * FCH
                        rps = jpp.tile([P_CH, FCH], DT, tag="mm", bufs=4,
                                       name="rps")
                        for kt in range(MT):
                            nc.tensor.matmul(
                                rps[:],
                                win_s[:, kt, DC + c * P_CH:DC + (c + 1) * P_CH],
                                xT[kt][:, f0:f0 + FCH],
                                start=(kt == 0), stop=(kt == MT - 1))
                        sg = jp.tile([P_CH, FCH], BF, tag="sg", bufs=2,
                                     name="sg")
                        nc.scalar.activation(sg[:], rps[:], AF.Sigmoid)
                        nc.vector.tensor_tensor(sres[c][:, f0:f0 + FCH],
                                                rps[:], sg[:], OP.mult)

            # ================= scan phase + out_proj (per batch) ===========
            with tc.tile_pool(name="scan_sb", bufs=1) as sp, \
                 tc.tile_pool(name="scan_ps", bufs=1, space="PSUM") as spp, \
                 tc.tile_pool(name="comb", bufs=1) as kp:

                def rep_tile():
                    return spp.tile([P, 2 * FCH], DT, tag="rep", bufs=3,
                                    name="rep")

                for b in range(cfg.B):
                    bo = b * L
                    for di, d in enumerate("fb"):
                        off = di * E
                        src = dbc_red0[d] if b == 0 else dbc_red[b]
                        soff = 0 if b == 0 else off
                        dt_sb = sp.tile([R, L], F32R, tag="dt", bufs=2,
                                        name=f"dt_{d}{b}")
                        nc.sync.dma_start(
                            dt_sb[:], src[soff:soff + R, :].bitcast(F32R))
                        bc_sb = sp.tile([2 * N, L], F32R, tag="bc", bufs=2,
                                        name=f"bc_{d}{b}")
                        nc.sync.dma_start(
                            bc_sb[:],
                            src[soff + R:soff + E, :].bitcast(F32R))

                        # B/C replicated across the 8-channel groups
                        brep = sp.tile([P, L], BF, tag="brep", bufs=2,
                                       name=f"brep{d}")
                        crep = sp.tile([P, L], BF, tag="crep", bufs=2,
                                       name=f"crep{d}")
                        for which, rep in ((0, brep), (1, crep)):
                            ps = rep_tile()
                            for lh in range(LCH):
                                o = lh * FCH
                                nc.tensor.matmul(
                                    ps[:, o:o + FCH],
                                    tsel_s[:, which, :],
                                    bc_sb[:, o:o + FCH],
                                    start=True, stop=True)
                            nc.scalar.copy(rep[:], ps[:])

                        # delta = softplus(dt @ WdtT + bdt); w = delta * u
                        delta = [sp.tile([P_CH, L], BF, tag=f"delta{c}",
                                         bufs=2, name=f"delta_{d}{c}")
                                 for c in range(CHT)]
                        w_s = [sp.tile([P_CH, L], BF, tag=f"w{c}", bufs=2,
                                       name=f"w_{d}{c}") for c in range(CHT)]
                        for c in range(CHT):
                            ps = rep_tile()
                            for lh in range(LCH):
                                o = lh * FCH
                                nc.tensor.matmul(
                                    ps[:, o:o + FCH],
                                    wdt_s[d][:, c * P_CH:(c + 1) * P_CH],
                                    dt_sb[:, o:o + FCH],
                                    start=True, stop=True)
                            spt = sp.tile([P_CH, L], DT, tag="spt", bufs=2,
                                          name="spt")
                            # softplus(x + bdt) = ln(1 + exp(x + bdt))
                            nc.scalar.activation(
                                spt[:], ps[:], AF.Exp,
                                bias=bdt_s[d][:P_CH, c:c + 1])
                            nc.scalar.activation(
                                delta[c][:], spt[:], AF.Ln, bias=1.0)
                            nc.vector.tensor_tensor(
                                w_s[c][:], delta[c][:],
                                u_bf[c][:, bo:bo + L], OP.mult)

                        for c in range(CHT):
                            hC = [None] * 16
                            for k in range(4):      # concurrent quad sets
                                dA, dBu = {}, {}
                                dpq, wpq = {}, {}
                                for i in range(4):
                                    g = 4 * i + k
                                    dpq[g] = rep_tile()
                                    for lh in range(LCH):
                                        q = lh * FCH
                                        nc.tensor.matmul(
                                            dpq[g][:, q:q + FCH],
                                            rall32_s[32 * i:32 * i + 32,
                                                     k, :],
                                            delta[c][32 * i:32 * i + 32,
                                                     q:q + FCH],
                                            start=True, stop=True,
                                            tile_position=(32 * i, 0))
                                for i in range(4):
                                    g = 4 * i + k
                                    j = c * 16 + g
                                    dA[g] = sp.tile([P, L], BF, tag="dA",
                                                    bufs=6, name="dA")
                                    nc.scalar.activation(
                                        dA[g][:], dpq[g][:], AF.Exp,
                                        scale=acol_s[d][:, j:j + 1])
                                for i in range(4):
                                    g = 4 * i + k
                                    wpq[g] = rep_tile()
                                    for lh in range(LCH):
                                        q = lh * FCH
                                        nc.tensor.matmul(
                                            wpq[g][:, q:q + FCH],
                                            rall32_s[32 * i:32 * i + 32,
                                                     k, :],
                                            w_s[c][32 * i:32 * i + 32,
                                                   q:q + FCH],
                                            start=True, stop=True,
                                            tile_position=(32 * i, 0))
                                for i in range(4):
                                    g = 4 * i + k
                                    dBu[g] = sp.tile([P, L], BF, tag="dBu",
                                                     bufs=6, name="dBu")
                                    if i == 0:
                                        nc.vector.tensor_tensor(
                                            dBu[g][:], wpq[g][:], brep[:],
                                            OP.mult)
                                    else:
                                        wsb = sp.tile([P, L], BF, tag="wsb",
                                                      bufs=3, name="wsb")
                                        nc.scalar.copy(wsb[:], wpq[g][:])
                                        nc.vector.tensor_tensor(
                                            dBu[g][:], wsb[:], brep[:],
                                            OP.mult)
                                for i in range(4):
                                    g = 4 * i + k
                                    h = sp.tile([P, L], BF, tag="h", bufs=10,
                                                name="h")
                                    if d == "f":
                                        nc.vector.tensor_tensor_scan(
                                            h[:], dA[g][:], dBu[g][:], 0.0,
                                            OP.mult, OP.add)
                                    else:
                                        nc.vector.tensor_tensor_scan(
                                            h[:, ::-1], dA[g][:, ::-1],
                                            dBu[g][:, ::-1],
                                            0.0, OP.mult, OP.add)
                                    hC[g] = sp.tile([P, L], BF, tag="hC",
                                                    bufs=17, name="hC")
                                    nc.vector.tensor_tensor(
                                        hC[g][:], h[:], crep[:], OP.mult)
                            # column-tiled y reduction over all 16 groups
                            y_ps = spp.tile([P_CH, L], DT, tag="y", bufs=1,
                                            name="y_ps")
                            for lh in range(LCH):
                                q = lh * FCH
                                for k in range(4):
                                    for i in range(4):
                                        g = 4 * i + k
                                        nc.tensor.matmul(
                                            y_ps[32 * i:32 * i + 32,
                                                 q:q + FCH],
                                            sall32_s[:, k, :],
                                            hC[g][:, q:q + FCH],
                                            start=(k == 0),
                                            stop=(k == 3 and d == "f"),
                                            tile_position=(0, 32 * i))
                                if d == "b":
                                    # add u*(fD+bD) into the accumulation
                                    for i in range(4):
                                        nc.tensor.matmul(
                                            y_ps[32 * i:32 * i + 32,
                                                 q:q + FCH],
                                            ddiag_s[:, c, i, :],
                                            u_bf[c][:, bo + q:bo + q + FCH],
                                            start=False, stop=True,
                                            tile_position=(0, 32 * i))
                            if d == "f":
                                nc.scalar.copy(y_f[c][:, bo:bo + L], y_ps[:])
                            else:
                                # fused combine:
                                # y = (y_f + y_b + u*(fD+bD)) * (0.5*silu(res))
                                # (the 0.5 is folded into W_out host-side)
                                ysl = y_f[c][:, bo:bo + L]
                                t1 = kp.tile([P_CH, L], BF, tag="t5", bufs=2,
                                             name="t1")
                                nc.vector.tensor_tensor(t1[:], y_ps[:],
                                                        ysl, OP.add)
                                nc.vector.tensor_tensor(
                                    ysl, t1[:], sres[c][:, bo:bo + L],
                                    OP.mult)

                    # out_proj + ReduceScatter for this batch
                    MFC = min(512, M)
                    for r in RS_OF_B[b]:
                        toff, sz = RS_CHUNKS[r]
                        for tr in range(sz // P):
                            t0 = toff + tr * P
                            ops = spp.tile([P, M], DT,
                                           tag=("rep" if b == 1 else "y"),
                                           bufs=(3 if b == 1 else 1),
                                           name="ops")
                            for mc in range(M // MFC):
                                o = mc * MFC
                                for c in range(CHT):
                                    nc.tensor.matmul(
                                        ops[:, o:o + MFC],
                                        y_f[c][:, t0:t0 + P],
                                        wout_s[:, c, o:o + MFC],
                                        start=(c == 0), stop=(c == CHT - 1))
                            ost = kp.tile([P, M], BF, tag="ost", bufs=3,
                                          name="ost")
                            nc.scalar.copy(ost[:], ops[:])
                            nc.sync.dma_start(
                                out_part[r][tr * P:(tr + 1) * P, :], ost[:])
                        nc.gpsimd.collective_compute(
                            "ReduceScatter", OP.add, replica_groups=rg,
                            ins=[out_part[r].opt()], outs=[out_rs[r].opt()])
                        nc.sync.dma_start(
                            out_d.ap()[toff // cfg.n_cores:
                                       (toff + sz) // cfg.n_cores, :],
                            out_rs[r][:])

    nc.compile()
    return nc


# --------------------------------------------------------------------------
# host side
# --------------------------------------------------------------------------

def host_prep(cfg: Cfg, inputs: dict) -> list[dict]:
    """Slice the full-model inputs into one input map per core."""
    P = 128
    f32 = np.float32

    def g(name):
        return np.asarray(inputs[name], f32)

    x = g("x").reshape(cfg.TOK, cfg.M)
    W_in = g("W_in")
    W_conv = g("W_conv").reshape(cfg.DI, cfg.KC)
    b_conv = g("b_conv")
    W_out = g("W_out")
    ident, rall32, t_sel, sall32 = build_consts(cfg)
    tsel_flat = t_sel.reshape(2 * cfg.N, 2 * P)
    sall32_flat = sall32.reshape(P, 4 * 32)
    rall32_flat = rall32.reshape(P, 4 * P)

    per = {}
    for d in "fb":
        per[d] = dict(
            A=-np.exp(g(d + "A_log")),            # (DI, N)
            D=g(d + "D"),
            Wx=g(d + "Wx"),                       # (E, DI)
            Wdt=g(d + "Wdt"),                     # (DI, R)
            bdt=g(d + "bdt"),
        )

    def col_layout(v):  # (DC,) -> (P_CH, CHT): [p, c] = v[c*P_CH + p]
        return np.ascontiguousarray(
            v.reshape(cfg.CHT, cfg.P_CH).T.astype(f32))

    def pad_p(a):  # pad partition dim up to 128
        if a.shape[0] == P:
            return np.ascontiguousarray(a.astype(f32))
        out = np.zeros((P,) + a.shape[1:], f32)
        out[:a.shape[0]] = a
        return out

    in_maps = []
    for core in range(cfg.n_cores):
        c0 = core * cfg.DC
        ch = slice(c0, c0 + cfg.DC)
        m = {
            "x": x,
            "winuT": np.ascontiguousarray(
                W_in[ch, :].T.astype(ml_dtypes.bfloat16)),
            "winrT": np.ascontiguousarray(
                W_in[cfg.DI + c0:cfg.DI + c0 + cfg.DC, :]
                .T.astype(ml_dtypes.bfloat16)),
            "wconv": pad_p(
                W_conv[ch].reshape(cfg.CHT, cfg.P_CH, cfg.KC)
                .transpose(1, 0, 2).reshape(cfg.P_CH, cfg.CHT * cfg.KC)),
            "bconv": pad_p(col_layout(b_conv[ch])),
            "woutT": np.ascontiguousarray(
                (W_out[:, ch].T * 0.5).astype(ml_dtypes.bfloat16)),
            "ident": ident,
            "ddiag": build_ddiag(
                cfg, (per["f"]["D"][ch] + per["b"]["D"][ch]).astype(f32)
            ).astype(ml_dtypes.bfloat16),
            "rall32": rall32_flat.astype(ml_dtypes.bfloat16),
            "tsel": tsel_flat,
            "sall32": sall32_flat.astype(ml_dtypes.bfloat16),
        }
        for d in "fb":
            pd = per[d]
            m[f"wx{d}T"] = np.ascontiguousarray(pd["Wx"][:, ch].T)
            m[f"wdt{d}T"] = np.ascontiguousarray(pd["Wdt"][ch, :].T)
            m[f"bdt{d}"] = pad_p(col_layout(pd["bdt"][ch]))
            # A columns: [p, j] = A[8j + p//16, p%16] (local channels)
            Ac = pd["A"][ch]                       # (DC, N)
            acol = np.empty((P, cfg.NT), f32)
            pidx = np.arange(P)
            for j in range(cfg.NT):
                acol[:, j] = Ac[8 * j + pidx // 16, pidx % 16]
            m[f"acol{d}"] = acol
        in_maps.append({k: np.ascontiguousarray(v) for k, v in m.items()})
    return in_maps


RS_CHUNKS_HOST = [(0, 512), (512, 512), (1024, 1024)]


def gather_out(cfg: Cfg, results: list[dict]) -> np.ndarray:
    out = np.empty((cfg.TOK, cfg.M), np.float32)
    for core in range(cfg.n_cores):
        shard = np.asarray(results[core]["out_rs"])  # (TOK//n_cores, M)
        cum = 0
        for toff, sz in RS_CHUNKS_HOST:
            sh = sz // cfg.n_cores
            out[toff + core * sh:toff + (core + 1) * sh, :] = \
                shard[cum:cum + sh, :]
            cum += sh
    return out.reshape(cfg.B, cfg.L, cfg.M).astype(np.float32)


def kernel(**inputs) -> np.ndarray:
    cfg = FULL
    from concourse.bass_utils import run_bass_kernel_spmd
    nc = build_program(cfg)
    in_maps = host_prep(cfg, inputs)
    res = run_bass_kernel_spmd(nc, in_maps, core_ids=list(range(cfg.n_cores)))
    return gather_out(cfg, res.results)


# revision 92
# speedup vs baseline: 1.1593x; 1.0056x over previous
"""Bidirectional Mamba block (in_proj -> depthwise causal conv -> SiLU ->
forward+backward S6 selective scan -> gated combine -> out_proj) as a
Trainium2 Bass/Tile SPMD kernel over 8 NeuronCores.

Sharding: tensor-parallel over d_inner (256 channels per core). The conv and
the S6 scans are channel-independent, so they need no communication.

Collectives are pipelined per batch element:
  * AllReduce(b) of both directions' partial x-projections dbc = u @ Wx^T
    ([192, 1024] fp32) is issued as soon as batch b's conv is done; AR(b0)
    overlaps the b1 front-end and the res projection, AR(b1) overlaps the
    b0 scan phase.
  * AR(b0) is further split by direction so the forward half (which gates
    the first scan) lands earlier.
  * The partial out-projection is cast to bf16 and ReduceScattered: two
    chunks for b0 (they overlap the b1 scan phase) but a single chunk for
    b1, whose only cost is launch+ring latency at the tail.  (More or
    uneven chunks measured slower: each extra exposed collective costs
    tens of microseconds of serialized launch/ring time.)

Scan layout: partitions = (8 channels x 16 states), free dim = L, one scan
instruction per (direction, batch, channel-group) on the DVE hardware scan
(fp32 internal state; the per-element feedback bubble pins it at ~2.2
cycles/element regardless of dtype).  The x transposes, in_proj and the
whole scan phase run in bf16 (rel err ~6e-3 vs the 2e-2 budget).

The entire dBu -> scan -> hC chain runs on the vector engine: for 3 of the
4 groups in a quad the scalar engine first evacuates the PE-produced w
replica to bf16 SBUF so the dBu multiply runs at the DVE 2x rate; the
fourth multiplies straight out of PSUM (1x) to keep the scalar engine
under its saturation point.  hC runs at the 2x bf16 rate.  Offloading
dBu/hC to GpSimd measures *slower* -- a GpSimd tensor op co-running with a
scan inflates the scan ~1.8us via SBUF port contention, more than the
offload saves.

PE array tiling: the delta/w replication matmuls have a true contraction of
8 rows, so four channel groups run concurrently in 32x128 row-tiles
(tile_position=(32i, 0), 32-row stationaries selecting 8 rows); the
per-group state reduction y = sum_n C*h has 8 output rows, so four groups
run concurrently in 128x32 column tiles (tile_position=(0, 32i)), and the
skip term u*(fD+bD) is folded into the same PSUM accumulation as a fifth,
diagonal column-tiled matmul.  Channel group g = 4i+k lives at partition
base 32i+8k, which places every concurrent quad {k, 4+k, 8+k, 12+k} in four
distinct array quadrants without any host-side channel permutation.
"""

import os
import sys

for _p in ("/opt/trn_rl_repo", "/root/.axon_site/_ro/trn_rl_repo"):
    if os.path.isdir(_p) and _p not in sys.path:
        sys.path.append(_p)

from dataclasses import dataclass

import ml_dtypes
import numpy as np

import concourse.bass as bass
import concourse.mybir as mybir
import concourse.tile as tile
from concourse import bacc

DT = mybir.dt.float32
F32R = mybir.dt.float32r
BF = mybir.dt.bfloat16
AF = mybir.ActivationFunctionType
OP = mybir.AluOpType


@dataclass(frozen=True)
class Cfg:
    n_cores: int = 8
    B: int = 2
    L: int = 1024
    M: int = 1024      # d_model
    DI: int = 2048     # d_inner
    N: int = 16        # d_state
    R: int = 64        # dt_rank
    KC: int = 4        # conv kernel

    @property
    def DC(self):  # channels per core
        return self.DI // self.n_cores

    @property
    def TOK(self):
        return self.B * self.L

    @property
    def P_CH(self):  # partitions per channel tile
        return min(128, self.DC)

    @property
    def CHT(self):  # channel tiles per core
        return self.DC // self.P_CH

    @property
    def NT(self):  # scan tiles per (dir, batch): 8 channels each
        return self.DC // 8

    @property
    def TPC(self):  # scan tiles per channel tile
        return self.P_CH // 8

    @property
    def FCH(self):  # matmul moving-dim chunk over tokens
        return min(512, self.L)

    @property
    def E(self):
        return self.R + 2 * self.N

    def check(self):
        assert self.DC % 8 == 0 and self.DC % self.P_CH == 0
        assert self.M % 128 == 0
        assert self.TOK % 128 == 0 and self.TOK % self.FCH == 0
        assert self.L % min(512, self.L) == 0
        assert self.N == 16 and self.TPC == 16


FULL = Cfg()


def build_consts(cfg: Cfg):
    """Selection matrices used as PE 'weights' (exact 0/1 values)."""
    P = 128
    ident = np.eye(P, dtype=np.float32)
    # rall32[32i+r, k, col] = 1 iff r in [8k, 8k+8) and col//16 == r-8k:
    # a 32-row stationary (32-aligned, as the PE tiling requires) that
    # replicates the 8 channel rows of group 4i+k into 128 (ch x state)
    # rows; the other 24 rows of the quadrant contribute zeros.
    rall32 = np.zeros((P, 4, P), np.float32)
    for p in range(P):
        r = p % 32
        for k in range(4):
            if 8 * k <= r < 8 * k + 8:
                rall32[p, k, (r - 8 * k) * 16:(r - 8 * k + 1) * 16] = 1.0
    # tsel[:, which, :]: out[p] = src[16*which + p%16]  (B/C replication)
    t_sel = np.zeros((2 * cfg.N, 2, P), np.float32)
    for which in range(2):
        for p in range(P):
            t_sel[cfg.N * which + p % 16, which, p] = 1.0
    # sall32[p, k, j] = 1 iff j == 8k + p//16: reduce the 16 state rows of
    # channel e into output row 8k+e of a 32-row column tile.
    sall32 = np.zeros((P, 4, 32), np.float32)
    for p in range(P):
        for k in range(4):
            sall32[p, k, 8 * k + p // 16] = 1.0
    # ddiag[p, c, i, j] = dsum[c*128+p] iff p == 32i+j: adds u*(fD+bD) into
    # the y accumulation as a 4-way column-tiled diagonal matmul.
    return ident, rall32, t_sel, sall32


def build_ddiag(cfg: Cfg, dsum_local: np.ndarray):
    P = 128
    ddiag = np.zeros((P, cfg.CHT, 4, 32), np.float32)
    for c in range(cfg.CHT):
        for p in range(P):
            i, j = p // 32, p % 32
            ddiag[p, c, i, j] = dsum_local[c * P + p]
    return ddiag.reshape(P, cfg.CHT * 4 * 32)


def build_program(cfg: Cfg) -> bass.Bass:
    cfg.check()
    P = 128
    TOK, L, M = cfg.TOK, cfg.L, cfg.M
    DC, CHT, P_CH = cfg.DC, cfg.CHT, cfg.P_CH
    FCH = cfg.FCH
    MT = M // P               # m tiles
    E, R, N = cfg.E, cfg.R, cfg.N
    LCH = L // FCH            # matmul chunks per batch
    TBB = L // P              # token blocks per batch

    nc = bacc.Bacc(
        "TRN2", target_bir_lowering=False, debug=False, num_devices=cfg.n_cores
    )

    # ---- kernel I/O ----
    x_d = nc.dram_tensor("x", [TOK, M], DT, kind="ExternalInput")
    winuT_d = nc.dram_tensor("winuT", [M, DC], BF, kind="ExternalInput")
    winrT_d = nc.dram_tensor("winrT", [M, DC], BF, kind="ExternalInput")
    wconv_d = nc.dram_tensor("wconv", [P, CHT * cfg.KC], DT, kind="ExternalInput")
    bconv_d = nc.dram_tensor("bconv", [P, CHT], DT, kind="ExternalInput")
    wxT_d = {d: nc.dram_tensor(f"wx{d}T", [DC, E], F32R, kind="ExternalInput")
             for d in "fb"}
    wdtT_d = {d: nc.dram_tensor(f"wdt{d}T", [R, DC], F32R, kind="ExternalInput")
              for d in "fb"}
    bdt_d = {d: nc.dram_tensor(f"bdt{d}", [P, CHT], DT, kind="ExternalInput")
             for d in "fb"}
    acol_d = {d: nc.dram_tensor(f"acol{d}", [P, CHT * cfg.TPC], DT,
                                kind="ExternalInput")
              for d in "fb"}
    woutT_d = nc.dram_tensor("woutT", [DC, M], BF, kind="ExternalInput")
    ident_d = nc.dram_tensor("ident", [P, P], DT, kind="ExternalInput")
    rall32_d = nc.dram_tensor("rall32", [P, 4 * P], BF, kind="ExternalInput")
    tsel_d = nc.dram_tensor("tsel", [2 * N, 2 * P], F32R, kind="ExternalInput")
    sall32_d = nc.dram_tensor("sall32", [P, 4 * 32], BF, kind="ExternalInput")
    ddiag_d = nc.dram_tensor("ddiag", [P, CHT * 4 * 32], BF,
                             kind="ExternalInput")

    out_d = nc.dram_tensor("out_rs", [TOK // cfg.n_cores, M], BF,
                           kind="ExternalOutput")

    rg = [list(range(cfg.n_cores))]
    cc_space = "Shared" if cfg.n_cores > 4 else "Local"
    # ReduceScatter chunk schedule (token offset, size): the final chunks are
    # smaller so the exposed tail after the last y is short.
    RS_CHUNKS = [(0, 512), (512, 512), (1024, 1024)]
    RS_OF_B = {0: [0, 1], 1: [2]}

    with tile.TileContext(nc) as tc:
        with tc.tile_pool(name="persist", bufs=1) as pp, \
             tc.tile_pool(name="dram", bufs=1, space="DRAM") as dp:

            # ---------- persistent SBUF (small weights) --------------------
            ident_s = pp.tile([P, P], DT)
            nc.sync.dma_start(ident_s[:], ident_d.ap())
            identb_s = pp.tile([P, P], BF)
            nc.scalar.copy(identb_s[:], ident_s[:])
            rall32_s = pp.tile([P, 4, P], BF)
            nc.sync.dma_start(rall32_s[:], rall32_d.ap().rearrange(
                "p (a b) -> p a b", a=4))
            tsel_s = pp.tile([2 * N, 2, P], F32R)
            nc.sync.dma_start(tsel_s[:], tsel_d.ap().rearrange(
                "k (a b) -> k a b", a=2))
            sall32_s = pp.tile([P, 4, 32], BF)
            nc.sync.dma_start(sall32_s[:], sall32_d.ap().rearrange(
                "p (a b) -> p a b", a=4))
            ddiag_s = pp.tile([P, CHT, 4, 32], BF)
            nc.sync.dma_start(ddiag_s[:], ddiag_d.ap().rearrange(
                "p (c a b) -> p c a b", c=CHT, a=4))
            wconv_s = pp.tile([P, CHT, cfg.KC], DT)
            nc.sync.dma_start(wconv_s[:], wconv_d.ap().rearrange(
                "p (c k) -> p c k", c=CHT))
            bconv_s = pp.tile([P, CHT], DT)
            nc.sync.dma_start(bconv_s[:], bconv_d.ap())
            wx_s, wdt_s, bdt_s, acol_s = {}, {}, {}, {}
            for d in "fb":
                wx_s[d] = pp.tile([P_CH, CHT, E], F32R, name=f"wx{d}_s")
                nc.sync.dma_start(wx_s[d][:], wxT_d[d].ap().rearrange(
                    "(c p) e -> p c e", p=P_CH))
                wdt_s[d] = pp.tile([R, DC], F32R, name=f"wdt{d}_s")
                nc.sync.dma_start(wdt_s[d][:], wdtT_d[d].ap())
                bdt_s[d] = pp.tile([P, CHT], DT, name=f"bdt{d}_s")
                nc.sync.dma_start(bdt_s[d][:], bdt_d[d].ap())
                acol_s[d] = pp.tile([P, CHT * cfg.TPC], DT, name=f"acol{d}_s")
                nc.sync.dma_start(acol_s[d][:], acol_d[d].ap())
            wout_s = pp.tile([P_CH, CHT, M], BF)

            u_c = [pp.tile([P_CH, TOK], F32R, name=f"u_c{c}") for c in range(CHT)]
            u_bf = [pp.tile([P_CH, TOK], BF, name=f"u_bf{c}") for c in range(CHT)]
            sres = [pp.tile([P_CH, TOK], BF, name=f"sres{c}")
                    for c in range(CHT)]
            y_f = [pp.tile([P_CH, TOK], BF, name=f"y_f{c}") for c in range(CHT)]

            # per-batch dbc partials/reductions (both directions merged)
            dbc_part = [dp.tile([2 * E, L], DT, name=f"dbc_part{b}")
                        for b in range(cfg.B)]
            dbc_red = [dp.tile([2 * E, L], DT, addr_space=cc_space,
                               name=f"dbc_red{b}") for b in range(cfg.B)]
            dbc_red0 = {d: dp.tile([E, L], DT, addr_space=cc_space,
                                   name=f"dbc_red0{d}") for d in "fb"}
            out_part = [dp.tile([sz, M], BF, name=f"out_part{r}")
                        for r, (_, sz) in enumerate(RS_CHUNKS)]
            out_rs = [dp.tile([sz // cfg.n_cores, M], BF, name=f"out_rs{r}")
                      for r, (_, sz) in enumerate(RS_CHUNKS)]

            # ================= front-end (per batch) + res-proj ============
            with tc.tile_pool(name="proj", bufs=1) as jp, \
                 tc.tile_pool(name="proj_ps", bufs=1, space="PSUM") as jpp:
                xT = [jp.tile([P, TOK], BF, name=f"xT{mt}") for mt in range(MT)]
                win_s = jp.tile([P, MT, 2 * DC], BF)
                nc.sync.dma_start(win_s[:, :, :DC], winuT_d.ap().rearrange(
                    "(a p) c -> p a c", p=P))
                nc.sync.dma_start(win_s[:, :, DC:], winrT_d.ap().rearrange(
                    "(a p) c -> p a c", p=P))

                upad = [[jp.tile([P_CH, cfg.KC - 1 + L], DT,
                                 name=f"upad{c}_{b}")
                         for b in range(cfg.B)] for c in range(CHT)]

                TPG = min(4, MT)
                for b in range(cfg.B):
                    bo = b * L
                    for half in range(LCH):
                        t0 = b * TBB + half * (FCH // P)
                        for tb in range(t0, t0 + FCH // P):
                            xsb = jp.tile([P, M], DT, tag="xsb", bufs=2,
                                          name="xsb")
                            nc.sync.dma_start(
                                xsb[:], x_d.ap()[tb * P:(tb + 1) * P, :])
                            xbb = jp.tile([P, M], BF, tag="xbb", bufs=2,
                                          name="xbb")
                            nc.scalar.copy(xbb[:], xsb[:])
                            for mg in range(MT // TPG):
                                tp_ps = jpp.tile([P, TPG * P], BF, tag="tp",
                                                 bufs=4, name="tp_ps")
                                for k in range(TPG):
                                    mt = mg * TPG + k
                                    nc.tensor.transpose(
                                        tp_ps[:, k * P:(k + 1) * P],
                                        xbb[:, mt * P:(mt + 1) * P],
                                        identb_s[:])
                                for k in range(TPG):
                                    mt = mg * TPG + k
                                    nc.vector.tensor_copy(
                                        xT[mt][:, tb * P:(tb + 1) * P],
                                        tp_ps[:, k * P:(k + 1) * P])
                        f0 = half * FCH
                        for c in range(CHT):
                            ups = jpp.tile([P_CH, FCH], DT, tag="mm", bufs=4,
                                           name="ups")
                            for kt in range(MT):
                                nc.tensor.matmul(
                                    ups[:],
                                    win_s[:, kt, c * P_CH:(c + 1) * P_CH],
                                    xT[kt][:, bo + f0:bo + f0 + FCH],
                                    start=(kt == 0), stop=(kt == MT - 1))
                            nc.scalar.copy(
                                upad[c][b][:, cfg.KC - 1 + f0:
                                           cfg.KC - 1 + f0 + FCH], ups[:])

                    for c in range(CHT):
                        nc.gpsimd.memset(upad[c][b][:, :cfg.KC - 1], 0.0)
                    # depthwise causal conv + SiLU, then the dbc partials,
                    # per 512-token chunk so the AllReduce input is complete
                    # (and the collective launched) as early as possible
                    for lh in range(LCH):
                        f0 = lh * FCH
                        for c in range(CHT):
                            acc = None
                            for k in range(cfg.KC):
                                nxt = jp.tile([P_CH, FCH], DT, tag="cacc",
                                              bufs=2, name="cacc")
                                tap = upad[c][b][:, f0 + k:f0 + k + FCH]
                                wk = wconv_s[:P_CH, c, k:k + 1]
                                if acc is None:
                                    nc.vector.tensor_scalar(
                                        nxt[:], tap, wk,
                                        bconv_s[:P_CH, c:c + 1],
                                        OP.mult, OP.add)
                                else:
                                    nc.vector.scalar_tensor_tensor(
                                        nxt[:], tap, wk, acc[:],
                                        OP.mult, OP.add)
                                acc = nxt
                            sg2 = jp.tile([P_CH, FCH], DT, tag="sg2", bufs=2,
                                          name="sg2")
                            nc.scalar.activation(sg2[:], acc[:], AF.Sigmoid)
                            nc.gpsimd.tensor_tensor(
                                u_c[c][:, bo + f0:bo + f0 + FCH], acc[:],
                                sg2[:], OP.mult)
                            nc.scalar.copy(
                                u_bf[c][:, bo + f0:bo + f0 + FCH],
                                u_c[c][:, bo + f0:bo + f0 + FCH].bitcast(DT))
                        for di, d in enumerate("fb"):
                            bps = jpp.tile([E, FCH], DT, tag="mm", bufs=4,
                                           name="bps")
                            for c in range(CHT):
                                nc.tensor.matmul(
                                    bps[:],
                                    wx_s[d][:, c, :],
                                    u_c[c][:, bo + f0:bo + f0 + FCH],
                                    start=(c == 0), stop=(c == CHT - 1))
                            bst = jp.tile([E, FCH], DT, tag="bst", bufs=3,
                                          name="bst")
                            nc.scalar.copy(bst[:], bps[:])
                            nc.sync.dma_start(
                                dbc_part[b][di * E:(di + 1) * E,
                                            f0:f0 + FCH], bst[:])
                    if b == 0:
                        # split by direction: the f half lands earlier so the
                        # scan phase starts sooner; the b half hides under it
                        nc.gpsimd.collective_compute(
                            "AllReduce", OP.add, replica_groups=rg,
                            ins=[dbc_part[b][:E, :].opt()],
                            outs=[dbc_red0["f"].opt()])
                        nc.gpsimd.collective_compute(
                            "AllReduce", OP.add, replica_groups=rg,
                            ins=[dbc_part[b][E:, :].opt()],
                            outs=[dbc_red0["b"].opt()])
                    else:
                        nc.gpsimd.collective_compute(
                            "AllReduce", OP.add, replica_groups=rg,
                            ins=[dbc_part[b].opt()], outs=[dbc_red[b].opt()])

                # res projection (overlaps the AllReduces)
                for c in range(CHT):
                    for fc in range(TOK // FCH):
                        f0 = fc * FCH
                        rps = jpp.tile([P_CH, FCH], DT, tag="mm", bufs=4,
                                       name="rps")
                        for kt in range(MT):
                            nc.tensor.matmul(
                                rps[:],
                                win_s[:, kt, DC + c * P_CH:DC + (c + 1) * P_CH],
                                xT[kt][:, f0:f0 + FCH],
                                start=(kt == 0), stop=(kt == MT - 1))
                        sg = jp.tile([P_CH, FCH], BF, tag="sg", bufs=2,
                                     name="sg")
                        nc.scalar.activation(sg[:], rps[:], AF.Sigmoid)
                        nc.vector.tensor_tensor(sres[c][:, f0:f0 + FCH],
                                                rps[:], sg[:], OP.mult)

            # wout is first consumed by out_proj(b0); loading it late keeps
            # the DMA queues clear for x at kernel start
            nc.sync.dma_start(wout_s[:], woutT_d.ap().rearrange(
                "(c p) m -> p c m", p=P_CH))

            # ================= scan phase + out_proj (per batch) ===========
            with tc.tile_pool(name="scan_sb", bufs=1) as sp, \
                 tc.tile_pool(name="scan_ps", bufs=1, space="PSUM") as spp, \
                 tc.tile_pool(name="comb", bufs=1) as kp:

                def rep_tile():
                    return spp.tile([P, 2 * FCH], DT, tag="rep", bufs=3,
                                    name="rep")

                for b in range(cfg.B):
                    bo = b * L
                    for di, d in enumerate("fb"):
                        off = di * E
                        src = dbc_red0[d] if b == 0 else dbc_red[b]
                        soff = 0 if b == 0 else off
                        dt_sb = sp.tile([R, L], F32R, tag="dt", bufs=2,
                                        name=f"dt_{d}{b}")
                        nc.sync.dma_start(
                            dt_sb[:], src[soff:soff + R, :].bitcast(F32R))
                        bc_sb = sp.tile([2 * N, L], F32R, tag="bc", bufs=2,
                                        name=f"bc_{d}{b}")
                        nc.sync.dma_start(
                            bc_sb[:],
                            src[soff + R:soff + E, :].bitcast(F32R))

                        # B/C replicated across the 8-channel groups
                        brep = sp.tile([P, L], BF, tag="brep", bufs=2,
                                       name=f"brep{d}")
                        crep = sp.tile([P, L], BF, tag="crep", bufs=2,
                                       name=f"crep{d}")
                        for which, rep in ((0, brep), (1, crep)):
                            ps = rep_tile()
                            for lh in range(LCH):
                                o = lh * FCH
                                nc.tensor.matmul(
                                    ps[:, o:o + FCH],
                                    tsel_s[:, which, :],
                                    bc_sb[:, o:o + FCH],
                                    start=True, stop=True)
                            nc.scalar.copy(rep[:], ps[:])

                        # delta = softplus(dt @ WdtT + bdt); w = delta * u
                        delta = [sp.tile([P_CH, L], BF, tag=f"delta{c}",
                                         bufs=2, name=f"delta_{d}{c}")
                                 for c in range(CHT)]
                        w_s = [sp.tile([P_CH, L], BF, tag=f"w{c}", bufs=2,
                                       name=f"w_{d}{c}") for c in range(CHT)]
                        for c in range(CHT):
                            ps = rep_tile()
                            for lh in range(LCH):
                                o = lh * FCH
                                nc.tensor.matmul(
                                    ps[:, o:o + FCH],
                                    wdt_s[d][:, c * P_CH:(c + 1) * P_CH],
                                    dt_sb[:, o:o + FCH],
                                    start=True, stop=True)
                            spt = sp.tile([P_CH, L], DT, tag="spt", bufs=2,
                                          name="spt")
                            # softplus(x + bdt) = ln(1 + exp(x + bdt))
                            nc.scalar.activation(
                                spt[:], ps[:], AF.Exp,
                                bias=bdt_s[d][:P_CH, c:c + 1])
                            nc.scalar.activation(
                                delta[c][:], spt[:], AF.Ln, bias=1.0)
                            nc.vector.tensor_tensor(
                                w_s[c][:], delta[c][:],
                                u_bf[c][:, bo:bo + L], OP.mult)

                        for c in range(CHT):
                            hC = [None] * 16
                            for k in range(4):      # concurrent quad sets
                                dA, dBu = {}, {}
                                dpq, wpq = {}, {}
                                for i in range(4):
                                    g = 4 * i + k
                                    dpq[g] = rep_tile()
                                    for lh in range(LCH):
                                        q = lh * FCH
                                        nc.tensor.matmul(
                                            dpq[g][:, q:q + FCH],
                                            rall32_s[32 * i:32 * i + 32,
                                                     k, :],
                                            delta[c][32 * i:32 * i + 32,
                                                     q:q + FCH],
                                            start=True, stop=True,
                                            tile_position=(32 * i, 0))
                                for i in range(4):
                                    g = 4 * i + k
                                    j = c * 16 + g
                                    dA[g] = sp.tile([P, L], BF, tag="dA",
                                                    bufs=6, name="dA")
                                    nc.scalar.activation(
                                        dA[g][:], dpq[g][:], AF.Exp,
                                        scale=acol_s[d][:, j:j + 1])
                                for i in range(4):
                                    g = 4 * i + k
                                    wpq[g] = rep_tile()
                                    for lh in range(LCH):
                                        q = lh * FCH
                                        nc.tensor.matmul(
                                            wpq[g][:, q:q + FCH],
                                            rall32_s[32 * i:32 * i + 32,
                                                     k, :],
                                            w_s[c][32 * i:32 * i + 32,
                                                   q:q + FCH],
                                            start=True, stop=True,
                                            tile_position=(32 * i, 0))
                                for i in range(4):
                                    g = 4 * i + k
                                    dBu[g] = sp.tile([P, L], BF, tag="dBu",
                                                     bufs=6, name="dBu")
                                    if i == 0:
                                        nc.vector.tensor_tensor(
                                            dBu[g][:], wpq[g][:], brep[:],
                                            OP.mult)
                                    else:
                                        wsb = sp.tile([P, L], BF, tag="wsb",
                                                      bufs=3, name="wsb")
                                        nc.scalar.copy(wsb[:], wpq[g][:])
                                        nc.vector.tensor_tensor(
                                            dBu[g][:], wsb[:], brep[:],
                                            OP.mult)
                                for i in range(4):
                                    g = 4 * i + k
                                    h = sp.tile([P, L], BF, tag="h", bufs=10,
                                                name="h")
                                    if d == "f":
                                        nc.vector.tensor_tensor_scan(
                                            h[:], dA[g][:], dBu[g][:], 0.0,
                                            OP.mult, OP.add)
                                    else:
                                        nc.vector.tensor_tensor_scan(
                                            h[:, ::-1], dA[g][:, ::-1],
                                            dBu[g][:, ::-1],
                                            0.0, OP.mult, OP.add)
                                    hC[g] = sp.tile([P, L], BF, tag="hC",
                                                    bufs=17, name="hC")
                                    nc.vector.tensor_tensor(
                                        hC[g][:], h[:], crep[:], OP.mult)
                            # column-tiled y reduction over all 16 groups
                            y_ps = spp.tile([P_CH, L], DT, tag="y", bufs=1,
                                            name="y_ps")
                            for lh in range(LCH):
                                q = lh * FCH
                                for k in range(4):
                                    for i in range(4):
                                        g = 4 * i + k
                                        nc.tensor.matmul(
                                            y_ps[32 * i:32 * i + 32,
                                                 q:q + FCH],
                                            sall32_s[:, k, :],
                                            hC[g][:, q:q + FCH],
                                            start=(k == 0),
                                            stop=(k == 3 and d == "f"),
                                            tile_position=(0, 32 * i))
                                if d == "b":
                                    # add u*(fD+bD) into the accumulation
                                    for i in range(4):
                                        nc.tensor.matmul(
                                            y_ps[32 * i:32 * i + 32,
                                                 q:q + FCH],
                                            ddiag_s[:, c, i, :],
                                            u_bf[c][:, bo + q:bo + q + FCH],
                                            start=False, stop=True,
                                            tile_position=(0, 32 * i))
                            if d == "f":
                                nc.scalar.copy(y_f[c][:, bo:bo + L], y_ps[:])
                            else:
                                # fused combine:
                                # y = (y_f + y_b + u*(fD+bD)) * (0.5*silu(res))
                                # (the 0.5 is folded into W_out host-side)
                                ysl = y_f[c][:, bo:bo + L]
                                t1 = kp.tile([P_CH, L], BF, tag="t5", bufs=2,
                                             name="t1")
                                nc.vector.tensor_tensor(t1[:], y_ps[:],
                                                        ysl, OP.add)
                                nc.vector.tensor_tensor(
                                    ysl, t1[:], sres[c][:, bo:bo + L],
                                    OP.mult)

                    # out_proj + ReduceScatter for this batch
                    MFC = min(512, M)
                    for r in RS_OF_B[b]:
                        toff, sz = RS_CHUNKS[r]
                        for tr in range(sz // P):
                            t0 = toff + tr * P
                            ops = spp.tile([P, M], DT,
                                           tag=("rep" if b == 1 else "y"),
                                           bufs=(3 if b == 1 else 1),
                                           name="ops")
                            for mc in range(M // MFC):
                                o = mc * MFC
                                for c in range(CHT):
                                    nc.tensor.matmul(
                                        ops[:, o:o + MFC],
                                        y_f[c][:, t0:t0 + P],
                                        wout_s[:, c, o:o + MFC],
                                        start=(c == 0), stop=(c == CHT - 1))
                            ost = kp.tile([P, M], BF, tag="ost", bufs=3,
                                          name="ost")
                            nc.scalar.copy(ost[:], ops[:])
                            nc.sync.dma_start(
                                out_part[r][tr * P:(tr + 1) * P, :], ost[:])
                        nc.gpsimd.collective_compute(
                            "ReduceScatter", OP.add, replica_groups=rg,
                            ins=[out_part[r].opt()], outs=[out_rs[r].opt()])
                        nc.sync.dma_start(
                            out_d.ap()[toff // cfg.n_cores:
                                       (toff + sz) // cfg.n_cores, :],
                            out_rs[r][:])

    nc.compile()
    return nc


# --------------------------------------------------------------------------
# host side
# --------------------------------------------------------------------------

def host_prep(cfg: Cfg, inputs: dict) -> list[dict]:
    """Slice the full-model inputs into one input map per core."""
    P = 128
    f32 = np.float32

    def g(name):
        return np.asarray(inputs[name], f32)

    x = g("x").reshape(cfg.TOK, cfg.M)
    W_in = g("W_in")
    W_conv = g("W_conv").reshape(cfg.DI, cfg.KC)
    b_conv = g("b_conv")
    W_out = g("W_out")
    ident, rall32, t_sel, sall32 = build_consts(cfg)
    tsel_flat = t_sel.reshape(2 * cfg.N, 2 * P)
    sall32_flat = sall32.reshape(P, 4 * 32)
    rall32_flat = rall32.reshape(P, 4 * P)

    per = {}
    for d in "fb":
        per[d] = dict(
            A=-np.exp(g(d + "A_log")),            # (DI, N)
            D=g(d + "D"),
            Wx=g(d + "Wx"),                       # (E, DI)
            Wdt=g(d + "Wdt"),                     # (DI, R)
            bdt=g(d + "bdt"),
        )

    def col_layout(v):  # (DC,) -> (P_CH, CHT): [p, c] = v[c*P_CH + p]
        return np.ascontiguousarray(
            v.reshape(cfg.CHT, cfg.P_CH).T.astype(f32))

    def pad_p(a):  # pad partition dim up to 128
        if a.shape[0] == P:
            return np.ascontiguousarray(a.astype(f32))
        out = np.zeros((P,) + a.shape[1:], f32)
        out[:a.shape[0]] = a
        return out

    in_maps = []
    for core in range(cfg.n_cores):
        c0 = core * cfg.DC
        ch = slice(c0, c0 + cfg.DC)
        m = {
            "x": x,
            "winuT": np.ascontiguousarray(
                W_in[ch, :].T.astype(ml_dtypes.bfloat16)),
            "winrT": np.ascontiguousarray(
                W_in[cfg.DI + c0:cfg.DI + c0 + cfg.DC, :]
                .T.astype(ml_dtypes.bfloat16)),
            "wconv": pad_p(
                W_conv[ch].reshape(cfg.CHT, cfg.P_CH, cfg.KC)
                .transpose(1, 0, 2).reshape(cfg.P_CH, cfg.CHT * cfg.KC)),
            "bconv": pad_p(col_layout(b_conv[ch])),
            "woutT": np.ascontiguousarray(
                (W_out[:, ch].T * 0.5).astype(ml_dtypes.bfloat16)),
            "ident": ident,
            "ddiag": build_ddiag(
                cfg, (per["f"]["D"][ch] + per["b"]["D"][ch]).astype(f32)
            ).astype(ml_dtypes.bfloat16),
            "rall32": rall32_flat.astype(ml_dtypes.bfloat16),
            "tsel": tsel_flat,
            "sall32": sall32_flat.astype(ml_dtypes.bfloat16),
        }
        for d in "fb":
            pd = per[d]
            m[f"wx{d}T"] = np.ascontiguousarray(pd["Wx"][:, ch].T)
            m[f"wdt{d}T"] = np.ascontiguousarray(pd["Wdt"][ch, :].T)
            m[f"bdt{d}"] = pad_p(col_layout(pd["bdt"][ch]))
            # A columns: [p, j] = A[8j + p//16, p%16] (local channels)
            Ac = pd["A"][ch]                       # (DC, N)
            acol = np.empty((P, cfg.NT), f32)
            pidx = np.arange(P)
            for j in range(cfg.NT):
                acol[:, j] = Ac[8 * j + pidx // 16, pidx % 16]
            m[f"acol{d}"] = acol
        in_maps.append({k: np.ascontiguousarray(v) for k, v in m.items()})
    return in_maps


RS_CHUNKS_HOST = [(0, 512), (512, 512), (1024, 1024)]


def gather_out(cfg: Cfg, results: list[dict]) -> np.ndarray:
    out = np.empty((cfg.TOK, cfg.M), np.float32)
    for core in range(cfg.n_cores):
        shard = np.asarray(results[core]["out_rs"])  # (TOK//n_cores, M)
        cum = 0
        for toff, sz in RS_CHUNKS_HOST:
            sh = sz // cfg.n_cores
            out[toff + core * sh:toff + (core + 1) * sh, :] = \
                shard[cum:cum + sh, :]
            cum += sh
    return out.reshape(cfg.B, cfg.L, cfg.M).astype(np.float32)


def kernel(**inputs) -> np.ndarray:
    cfg = FULL
    from concourse.bass_utils import run_bass_kernel_spmd
    nc = build_program(cfg)
    in_maps = host_prep(cfg, inputs)
    res = run_bass_kernel_spmd(nc, in_maps, core_ids=list(range(cfg.n_cores)))
    return gather_out(cfg, res.results)
